# revision 1
# baseline (speedup 1.0000x reference)
"""Multi-headed attention (B=2, S=2048, D=768, H=12) on 8 TRN2 NeuronCores.

Sharding: data parallel on batch x tensor parallel on heads. Core c handles
batch c//4 and heads 3*(c%4) .. 3*(c%4)+2. Each core computes its partial
output projection [S, D]; the host sums the 4 partials per batch.

Key-position compaction: the mask is per key position only ([B,1,1,S],
values 0/1). Masked keys contribute exp(-1e9) == 0.0 exactly (fp32
underflow) to every softmax row, so the host drops masked key/value
positions before projection and pads to a multiple of 128; padded rows get
a -1e9 additive bias on the scores (same underflow-to-zero as the
reference's where(mask==0, -1e9, scores)). This is exact, not approximate.

Softmax runs without max-subtraction: scores ~ N(0,1) after the 1/sqrt(dk)
scale, so exp() cannot overflow; the reference's max-subtraction only
shifts numerator and denominator by a common factor.

On-device layouts (per core):
  qT [e_local, s]   e_local = 3 local heads x 64 = 192, stored as a
                    [128, 2048] pair tile (heads 0,1) + [64, 2048] tile
  kT [e_local, kpos] same split, kpos compacted+padded to S_pad
  v_aug [128, KB*3*65] - per (kblock, head): 64 v columns + a ones column
                    (the ones column makes the PV matmul also produce the
                    softmax denominator as row 64 of the PSUM tile)
  scores are computed transposed, sT[kpos, q], so the pad-bias is a
  per-partition scalar and exp() needs a single ScalarE pass per tile.

All matmul operands are bitcast to float32r (full-rate fp32 on the PE at
moving-dim >= 256; plain fp32 runs at 1/4 rate).
"""

import sys

for _p in ("/opt/trn_rl_repo",):
    if _p not in sys.path:
        sys.path.insert(0, _p)

import numpy as np

import concourse.bacc as bacc
import concourse.mybir as mybir
import concourse.tile as tile

B, S, D, H = 2, 2048, 768, 12
DK = D // H          # 64
NH = 3               # heads per core
E = NH * DK          # 192 local e width
N_CORES = 8
QN = 512             # q tile (moving free dim)
QC = S // QN         # 4
DCH = D // 128       # 6 contraction chunks for the projections
NEG = -1.0e9

F32 = mybir.dt.float32
F32R = mybir.dt.float32r


def _r(ap):
    """Bitcast a float32 AP to float32r (unused; tensors are native f32r)."""
    return ap.bitcast(F32R)


def _build_program(kb: int):
    """Build the single-core SPMD program for KB key blocks of 128."""
    sk = kb * 128
    nc = bacc.Bacc("TRN2", target_bir_lowering=False, debug=False)

    xq = nc.dram_tensor("xq_t", [D, S], F32R, kind="ExternalInput").ap()
    xk = nc.dram_tensor("xk_t", [D, sk], F32R, kind="ExternalInput").ap()
    xv = nc.dram_tensor("xv_t", [D, sk], F32R, kind="ExternalInput").ap()
    wq = nc.dram_tensor("wq_t", [D, E], F32R, kind="ExternalInput").ap()
    wk = nc.dram_tensor("wk_t", [D, E], F32R, kind="ExternalInput").ap()
    wv = nc.dram_tensor("wv_t", [D, 256], F32R, kind="ExternalInput").ap()
    wo = nc.dram_tensor("wo_t", [E, D], F32R, kind="ExternalInput").ap()
    bqk = nc.dram_tensor("bqk", [E, 2], F32, kind="ExternalInput").ap()
    mb = nc.dram_tensor("maskbias", [128, kb], F32, kind="ExternalInput").ap()
    ones_in = nc.dram_tensor("ones_in", [128, 64], F32R, kind="ExternalInput").ap()
    out = nc.dram_tensor("out", [S, D], F32, kind="ExternalOutput").ap()

    with tile.TileContext(nc) as tc:
        with (
            tc.tile_pool(name="resident", bufs=1) as res,
            tc.tile_pool(name="eT", bufs=6) as etp,
            tc.tile_pool(name="small", bufs=4) as small,
            tc.tile_pool(name="ucopy", bufs=4) as ucp,
        ):
            # ---- resident SBUF ----
            qTp = res.tile([128, S], F32R, tag="qTp")     # heads 0,1
            qTs = res.tile([64, S], F32R, tag="qTs")      # head 2
            kTp = res.tile([128, sk], F32R, tag="kTp")
            kTs = res.tile([64, sk], F32R, tag="kTs")
            v_aug = res.tile([128, kb * NH * 65], F32R, tag="vaug")
            woA = res.tile([128, D], F32R, tag="woA")
            woB = res.tile([64, D], F32R, tag="woB")
            mbt = res.tile([128, kb], F32, tag="mb")
            bqkA = res.tile([128, 2], F32, tag="bqkA")
            bqkB = res.tile([64, 2], F32, tag="bqkB")
            ones = res.tile([1, 64], F32R, tag="ones")
            wq_sb = res.tile([128, DCH * E], F32R, tag="wq")
            wk_sb = res.tile([128, DCH * E], F32R, tag="wk")
            wv_sb = res.tile([128, DCH * 256], F32R, tag="wv")

            nc.sync.dma_start(out=ones[:], in_=ones_in[0:1, :])
            nc.sync.dma_start(
                out=v_aug[:].rearrange("p (g c) -> p g c", c=65)[:, :, 64:65],
                in_=ones_in[:, 0:kb * NH].rearrange("p (g o) -> p g o", o=1),
            )
            nc.sync.dma_start(out=woA[:], in_=wo[0:128, :])
            nc.sync.dma_start(out=woB[:], in_=wo[128:192, :])
            nc.sync.dma_start(out=mbt[:], in_=mb[:, :])
            nc.sync.dma_start(out=bqkA[:], in_=bqk[0:128, :])
            nc.sync.dma_start(out=bqkB[:], in_=bqk[128:192, :])
            for dc in range(DCH):
                nc.sync.dma_start(
                    out=wq_sb[:, dc * E:(dc + 1) * E], in_=wq[dc * 128:(dc + 1) * 128, :]
                )
                nc.sync.dma_start(
                    out=wk_sb[:, dc * E:(dc + 1) * E], in_=wk[dc * 128:(dc + 1) * 128, :]
                )
                nc.sync.dma_start(
                    out=wv_sb[:, dc * 256:(dc + 1) * 256],
                    in_=wv[dc * 128:(dc + 1) * 128, :],
                )

            exp_f = mybir.ActivationFunctionType.Exp

            # ---- phase P: projections ----
            with (
                tc.tile_pool(name="xin", bufs=7) as xin,
                tc.tile_pool(name="proj_ps", bufs=4, space="PSUM") as proj_ps,
            ):
                for which, xdram, w_sb, scols, dq, ds_ in (
                    ("q", xq, wq_sb, S, (qTp, qTs), 0),
                    ("k", xk, wk_sb, sk, (kTp, kTs), 1),
                ):
                    xch = [
                        xin.tile([128, scols], F32R, tag="xch", name=f"xch_{which}{dc}")
                        for dc in range(DCH)
                    ]
                    for dc in range(DCH):
                        nc.sync.dma_start(
                            out=xch[dc][:], in_=xdram[dc * 128:(dc + 1) * 128, :]
                        )
                    pair, single = dq
                    for ec, ew in ((0, 128), (128, 64)):
                        for sc in range(0, scols, QN):
                            sw = min(QN, scols - sc)
                            ps = proj_ps.tile([128, QN], F32, tag="pp")
                            for dc in range(DCH):
                                nc.tensor.matmul(
                                    ps[:ew, :sw],
                                    w_sb[:, dc * E + ec:dc * E + ec + ew],
                                    xch[dc][:, sc:sc + sw],
                                    start=(dc == 0),
                                    stop=(dc == DCH - 1),
                                )
                            if ec == 0:
                                nc.vector.tensor_scalar_add(
                                    pair[:, sc:sc + sw], ps[:128, :sw],
                                    bqkA[:, ds_:ds_ + 1],
                                )
                            else:
                                nc.vector.tensor_scalar_add(
                                    single[:, sc:sc + sw], ps[:64, :sw],
                                    bqkB[:, ds_:ds_ + 1],
                                )

                # ---- v projection (natural layout, into v_aug) ----
                xvch = [
                    xin.tile([128, sk], F32R, tag="xch", name=f"xch_v{dc}")
                    for dc in range(DCH)
                ]
                for dc in range(DCH):
                    nc.sync.dma_start(
                        out=xvch[dc][:], in_=xv[dc * 128:(dc + 1) * 128, :]
                    )
                for sb in range(kb):
                    ps = proj_ps.tile([128, QN], F32, tag="pp")
                    for dc in range(DCH):
                        nc.tensor.matmul(
                            ps[:, :256],
                            xvch[dc][:, sb * 128:(sb + 1) * 128],
                            wv_sb[:, dc * 256:(dc + 1) * 256],
                            start=(dc == 0),
                            stop=(dc == DCH - 1),
                        )
                    for h in range(NH):
                        off = (sb * NH + h) * 65
                        nc.vector.tensor_copy(
                            v_aug[:, off:off + 64], ps[:, h * 64:(h + 1) * 64]
                        )

            # ---- phase A: attention ----
            xTAj = [
                res.tile([128, QN], F32R, tag=f"xTA{j}", name=f"xTA{j}")
                for j in range(QC)
            ]
            xTBj = [
                res.tile([64, QN], F32R, tag=f"xTB{j}", name=f"xTB{j}")
                for j in range(QC)
            ]
            with (
                tc.tile_pool(name="sT_ps", bufs=4, space="PSUM") as st_ps,
                tc.tile_pool(name="u_ps", bufs=3, space="PSUM") as u_ps,
                tc.tile_pool(name="bc_ps", bufs=1, space="PSUM") as bc_ps,
            ):
                for j in range(QC):
                    for h in range(NH):
                        if h < 2:
                            k_l = kTp[h * 64:(h + 1) * 64, :]
                            q_l = qTp[h * 64:(h + 1) * 64, :]
                        else:
                            k_l = kTs[:, :]
                            q_l = qTs[:, :]
                        u = u_ps.tile([65, QN], F32, tag="u")
                        for b_ in range(kb):
                            st = st_ps.tile([128, QN], F32, tag="st")
                            nc.tensor.matmul(
                                st[:, :],
                                k_l[:, b_ * 128:(b_ + 1) * 128],
                                q_l[:, j * QN:(j + 1) * QN],
                                start=True,
                                stop=True,
                            )
                            et = etp.tile([128, QN], F32R, tag="et")
                            nc.scalar.activation(
                                et[:, :], st[:, :], exp_f,
                                bias=mbt[:, b_:b_ + 1], scale=0.125,
                            )
                            nc.tensor.matmul(
                                u[:, :],
                                v_aug[:, (b_ * NH + h) * 65:(b_ * NH + h) * 65 + 65],
                                et[:, :],
                                start=(b_ == 0),
                                stop=(b_ == kb - 1),
                            )
                        rec = small.tile([1, QN], F32, tag="rec")
                        nc.vector.reciprocal(rec[:, :], u[64:65, :])
                        recr = small.tile([1, QN], F32R, tag="recr")
                        nc.vector.tensor_copy(recr[:, :], rec[:, :])
                        bc = bc_ps.tile([64, QN], F32, tag="bc")
                        nc.tensor.matmul(
                            bc[:, :], ones[:, :], recr[:, :],
                            start=True, stop=True,
                        )
                        uc = ucp.tile([64, QN], F32, tag="uc")
                        nc.vector.tensor_copy(uc[:, :], u[0:64, :])
                        xdst = (
                            xTAj[j][h * 64:(h + 1) * 64, :]
                            if h < 2
                            else xTBj[j][:, :]
                        )
                        nc.vector.tensor_mul(xdst, uc[:, :], bc[:, :])

            # ---- phase O: output projection ----
            with tc.tile_pool(name="out_ps", bufs=4, space="PSUM") as out_ps:
                for qb in range(S // 128):
                    ot = ucp.tile([128, D], F32, tag="ot")
                    for e0, ew in ((0, 512), (512, 256)):
                        ps = out_ps.tile([128, 512], F32, tag="op")
                        jq, cq = qb // 4, (qb % 4) * 128
                        nc.tensor.matmul(
                            ps[:, :ew],
                            xTAj[jq][:, cq:cq + 128],
                            woA[:, e0:e0 + ew],
                            start=True,
                            stop=False,
                        )
                        nc.tensor.matmul(
                            ps[:, :ew],
                            xTBj[jq][:, cq:cq + 128],
                            woB[:, e0:e0 + ew],
                            start=False,
                            stop=True,
                        )
                        nc.vector.tensor_copy(ot[:, e0:e0 + ew], ps[:, :ew])
                    nc.sync.dma_start(
                        out=out[qb * 128:(qb + 1) * 128, :], in_=ot[:, :]
                    )

    nc.compile()
    return nc


_PROGRAM_CACHE: dict[int, object] = {}


def _get_program(kb: int):
    if kb not in _PROGRAM_CACHE:
        _PROGRAM_CACHE[kb] = _build_program(kb)
    return _PROGRAM_CACHE[kb]


def _prep_inputs(query, key, value, mask, Wq, bq, Wk, bk, Wv, bv, Wo, bo):
    """Host-side shard prep. Returns (in_maps, meta)."""
    f32 = np.float32
    valid = [np.nonzero(mask[b, 0, 0, :] != 0)[0] for b in range(B)]
    s_valid = max((len(v) for v in valid), default=1)
    s_pad = max(128, -(-s_valid // 128) * 128)
    kb = s_pad // 128

    per_batch = []
    for b in range(B):
        vi = valid[b]
        xq_t = np.ascontiguousarray(query[b].T, dtype=f32)
        xk_c = np.zeros((s_pad, D), dtype=f32)
        xv_c = np.zeros((s_pad, D), dtype=f32)
        xk_c[: len(vi)] = key[b][vi]
        xv_c[: len(vi)] = value[b][vi]
        mbias = np.full(s_pad, NEG, dtype=f32)
        mbias[: len(vi)] = 0.0
        per_batch.append(
            dict(
                xq_t=xq_t,
                xk_t=np.ascontiguousarray(xk_c.T),
                xv_t=np.ascontiguousarray(xv_c.T),
                maskbias=np.ascontiguousarray(mbias.reshape(kb, 128).T),
            )
        )

    in_maps = []
    for c in range(N_CORES):
        b = c // 4
        h0 = NH * (c % 4)
        sl = slice(h0 * DK, (h0 + NH) * DK)
        wv_t = np.zeros((D, 256), dtype=f32)
        wv_t[:, :E] = Wv[sl, :].T
        bqk_ = np.stack([bq[sl], bk[sl]], axis=1).astype(f32)
        in_maps.append(
            dict(
                per_batch[b],
                wq_t=np.ascontiguousarray(Wq[sl, :].T, dtype=f32),
                wk_t=np.ascontiguousarray(Wk[sl, :].T, dtype=f32),
                wv_t=wv_t,
                wo_t=np.ascontiguousarray(Wo[:, sl].T, dtype=f32),
                bqk=np.ascontiguousarray(bqk_),
                ones_in=np.ones((128, 64), dtype=f32),
            )
        )
    return in_maps, kb


def kernel(query, key, value, mask, Wq, bq, Wk, bk, Wv, bv, Wo, bo):
    from concourse.bass_utils import run_bass_kernel_spmd

    query = np.asarray(query, dtype=np.float32)
    key = np.asarray(key, dtype=np.float32)
    value = np.asarray(value, dtype=np.float32)
    mask = np.asarray(mask)
    Wq, Wk, Wv, Wo = (np.asarray(a, dtype=np.float32) for a in (Wq, Wk, Wv, Wo))
    bq, bk, bv, bo = (np.asarray(a, dtype=np.float32) for a in (bq, bk, bv, bo))

    in_maps, kb = _prep_inputs(
        query, key, value, mask, Wq, bq, Wk, bk, Wv, bv, Wo, bo
    )
    nc = _get_program(kb)
    res = run_bass_kernel_spmd(nc, in_maps, core_ids=list(range(N_CORES)))

    out = np.zeros((B, S, D), dtype=np.float32)
    for c in range(N_CORES):
        out[c // 4] += res.results[c]["out"]
    # bv folds into the output as (sum_k p == 1) -> + bv @ Wo.T; bo is a plain
    # output bias. Both are zero for this problem's inputs; keep exactness for
    # any input without on-device cost.
    if np.any(bv) or np.any(bo):
        out += (bv @ Wo.T + bo)[None, None, :]
    return out



# revision 19
# speedup vs baseline: 1.8021x; 1.8021x over previous
"""Multi-headed attention (B=2, S=2048, D=768, H=12) on 8 TRN2 NeuronCores.

Sharding: data parallel on batch x tensor parallel on heads. Core c handles
batch c//4 and heads 3*(c%4) .. 3*(c%4)+2. Each core computes its partial
output projection [S, D]; the host sums the 4 partials per batch.

Key-position compaction: the mask is per key position only ([B,1,1,S],
values 0/1). The host drops masked key/value positions before projection and
pads to a multiple of 128. Pad positions need no score bias at all: their v
rows are zero (zero-padded xv) and their entry in the ones-column of v_aug
is zero, so they contribute exp(score)*0 = 0 to both the softmax numerator
and denominator - exactly like the reference's where(mask==0,-1e9,scores).

Softmax runs without max-subtraction: scores ~ N(0,1) after the 1/sqrt(dk)
scale (folded into Wq on the host), so exp() cannot overflow.

All matmuls run in bf16 (fp32 PSUM accumulate). The exp is split between
the Scalar engine (true exp) and the Vector engine (Schraudolph bit-trick:
i16 = int16(a*s + b) reinterpreted as bf16), which roughly balances the two
engines' PSUM-drain/normalize workloads; PSUM can only be read out through
those two engines, so their combined throughput is the design constraint.

On-device layouts (per core):
  qT [e_local, s]   e_local = 3 local heads x 64 = 192, stored as a
                    [128, 2048] pair tile (heads 0,1) + [64, 2048] tile
  kT [e_local, kpos] same split, kpos compacted+padded to S_pad
  v_aug [128, KB*3*66] - per (kblock, head): 64 v columns + a ones column
                    + 1 pad (66 keeps each group 4-byte aligned in SBUF)
                    (the ones column makes the PV matmul also produce the
                    softmax denominator as row 64 of the PSUM tile; it is
                    zeroed for pad rows)
  scores are computed transposed, sT[kpos, q].
"""

import sys

for _p in ("/opt/trn_rl_repo",):
    if _p not in sys.path:
        sys.path.insert(0, _p)

import numpy as np

import concourse.bacc as bacc
import concourse.mybir as mybir
import concourse.tile as tile

B, S, D, H = 2, 2048, 768, 12
DK = D // H          # 64
NH = 3               # heads per core
E = NH * DK          # 192 local e width
N_CORES = 8
QN = 512             # q tile (PSUM bank = 512 fp32)
QC = S // QN         # 4
DCH = D // 128       # 6 contraction chunks for the projections

F32 = mybir.dt.float32
F32R = mybir.dt.float32r
BF16 = mybir.dt.bfloat16
I16 = mybir.dt.int16

# Schraudolph exp in bf16 bit-space: bf16(2^(s/ln2)) ~= int16(s*A + B).
# A = 2^7/ln2; B = 127*2^7 + C with C tuned to center the piecewise-linear
# approximation of 2^frac (min-RMS correction ~ -0.0573*128).
EXP_A = 128.0 / np.log(2.0)
EXP_B = 16256.0 - 7.33

# Which kblock indices use the DVE Schraudolph exp (rest: ScalarE true exp).
# Chosen to balance ScalarE vs VectorE total busy time.
DVE_EXP_EVERY = 2   # b_ % DVE_EXP_EVERY == DVE_EXP_PHASE -> DVE
DVE_EXP_PHASE = 1


def _dve_exp(b_: int) -> bool:
    return b_ % DVE_EXP_EVERY == DVE_EXP_PHASE


DEBUG_TAPS = False


def _build_program(kb: int):
    """Build the single-core SPMD program for KB key blocks of 128."""
    sk = kb * 128
    nc = bacc.Bacc("TRN2", target_bir_lowering=False, debug=False)

    xq = nc.dram_tensor("xq_t", [D, S], BF16, kind="ExternalInput").ap()
    xk = nc.dram_tensor("xk_t", [D, sk], BF16, kind="ExternalInput").ap()
    xv = nc.dram_tensor("xv_t", [D, sk], BF16, kind="ExternalInput").ap()
    wq = nc.dram_tensor("wq_t", [D, E], BF16, kind="ExternalInput").ap()
    wk = nc.dram_tensor("wk_t", [D, E], BF16, kind="ExternalInput").ap()
    wv = nc.dram_tensor("wv_t", [D, E], BF16, kind="ExternalInput").ap()
    wo = nc.dram_tensor("wo_t", [E, D], BF16, kind="ExternalInput").ap()
    bqk = nc.dram_tensor("bqk", [E, 2], F32, kind="ExternalInput").ap()
    vones = nc.dram_tensor("vones", [128, kb * NH * 2], BF16, kind="ExternalInput").ap()
    out = nc.dram_tensor("out", [S, D], F32, kind="ExternalOutput").ap()
    if DEBUG_TAPS:
        qT_dbg = nc.dram_tensor("qT_dbg", [128, S], BF16, kind="ExternalOutput").ap()
        kT_dbg = nc.dram_tensor("kT_dbg", [128, sk], BF16, kind="ExternalOutput").ap()
        va_dbg = nc.dram_tensor("va_dbg", [128, kb * NH * 66], BF16, kind="ExternalOutput").ap()
        et_dbg = nc.dram_tensor("et_dbg", [128, QN], BF16, kind="ExternalOutput").ap()
        rec_dbg = nc.dram_tensor("rec_dbg", [1, QN], F32, kind="ExternalOutput").ap()
        bcs_dbg = nc.dram_tensor("bcs_dbg", [64, QN], F32, kind="ExternalOutput").ap()
        x_dbg = nc.dram_tensor("x_dbg", [128, QN], BF16, kind="ExternalOutput").ap()

    exp_f = mybir.ActivationFunctionType.Exp
    ident_f = mybir.ActivationFunctionType.Identity

    with tile.TileContext(nc) as tc:
        # partition_broadcast is a custom GpSimd ucode op; its library must
        # be resident on the Q7 cores before first use.
        from concourse import library_config

        nc.gpsimd.load_library(library_config.attn)
        with (
            tc.tile_pool(name="resident", bufs=1) as res,
            tc.tile_pool(name="eT", bufs=18) as etp,
            tc.tile_pool(name="small", bufs=4) as small,
            tc.tile_pool(name="ocopy", bufs=4) as ocp,
        ):
            # ---- resident SBUF ----
            qTp = res.tile([128, S], BF16, tag="qTp")     # heads 0,1
            qTs = res.tile([64, S], BF16, tag="qTs")      # head 2
            kTp = res.tile([128, sk], BF16, tag="kTp")
            kTs = res.tile([64, sk], BF16, tag="kTs")
            v_aug = res.tile([128, kb * NH * 66], BF16, tag="vaug")
            woA = res.tile([128, D], BF16, tag="woA")
            woB = res.tile([64, D], BF16, tag="woB")
            bqkA = res.tile([128, 2], F32, tag="bqkA")
            bqkB = res.tile([64, 2], F32, tag="bqkB")
            wq_sb = res.tile([128, DCH * E], BF16, tag="wq")
            wk_sb = res.tile([128, DCH * E], BF16, tag="wk")
            wv_sb = res.tile([128, DCH * E], BF16, tag="wv")
            xkc = [
                res.tile([128, sk], BF16, tag=f"xkc{dc}", name=f"xkc{dc}")
                for dc in range(DCH)
            ]
            xvc = [
                res.tile([128, sk], BF16, tag=f"xvc{dc}", name=f"xvc{dc}")
                for dc in range(DCH)
            ]
            xqc = [
                res.tile([128, S], BF16, tag=f"xqc{dc}", name=f"xqc{dc}")
                for dc in range(DCH)
            ]
            xTA = [
                res.tile([128, QN], BF16, tag=f"xTA{j}", name=f"xTA{j}")
                for j in range(S // QN)
            ]
            xTB = [
                res.tile([64, QN], BF16, tag=f"xTB{j}", name=f"xTB{j}")
                for j in range(S // QN)
            ]

            # ---- input DMAs (k/v first so attention prerequisites land early)
            for dc in range(DCH):
                nc.sync.dma_start(
                    out=wk_sb[:, dc * E:(dc + 1) * E],
                    in_=wk[dc * 128:(dc + 1) * 128, :],
                )
            nc.sync.dma_start(out=bqkA[:], in_=bqk[0:128, :])
            nc.sync.dma_start(out=bqkB[:], in_=bqk[128:192, :])
            for dc in range(DCH):
                nc.sync.dma_start(out=xkc[dc][:], in_=xk[dc * 128:(dc + 1) * 128, :])
            for dc in range(DCH):
                nc.sync.dma_start(
                    out=wv_sb[:, dc * E:(dc + 1) * E],
                    in_=wv[dc * 128:(dc + 1) * 128, :],
                )
            for dc in range(DCH):
                nc.sync.dma_start(out=xvc[dc][:], in_=xv[dc * 128:(dc + 1) * 128, :])
            nc.sync.dma_start(
                out=v_aug[:].rearrange("p (g c) -> p g c", c=66)[:, :, 64:66],
                in_=vones[:, :].rearrange("p (g o) -> p g o", o=2),
            )
            for dc in range(DCH):
                nc.sync.dma_start(
                    out=wq_sb[:, dc * E:(dc + 1) * E],
                    in_=wq[dc * 128:(dc + 1) * 128, :],
                )
            for dc in range(DCH):
                nc.sync.dma_start(out=xqc[dc][:], in_=xq[dc * 128:(dc + 1) * 128, :])
            nc.sync.dma_start(out=woA[:], in_=wo[0:128, :])
            nc.sync.dma_start(out=woB[:], in_=wo[128:192, :])

            # ---- phase P: projections ----
            with tc.tile_pool(name="proj_ps", bufs=4, space="PSUM") as proj_ps:
                # k projection -> kT (bias via ScalarE Identity)
                for ec, ew in ((0, 128), (128, 64)):
                    dstk = kTp if ec == 0 else kTs
                    bk_ap = (bqkA if ec == 0 else bqkB)
                    for sc in range(0, sk, QN):
                        sw = min(QN, sk - sc)
                        ps = proj_ps.tile([128, QN], F32, tag="pp")
                        for dc in range(DCH):
                            nc.tensor.matmul(
                                ps[:ew, :sw],
                                wk_sb[:, dc * E + ec:dc * E + ec + ew],
                                xkc[dc][:, sc:sc + sw],
                                start=(dc == 0),
                                stop=(dc == DCH - 1),
                            )
                        nc.scalar.activation(
                            dstk[:, sc:sc + sw], ps[:ew, :sw], ident_f,
                            bias=bk_ap[:ew, 1:2],
                        )

                # v projection (natural [kpos, e] layout, into v_aug)
                for sb in range(kb):
                    ps = proj_ps.tile([128, QN], F32, tag="pp")
                    for dc in range(DCH):
                        nc.tensor.matmul(
                            ps[:, :E],
                            xvc[dc][:, sb * 128:(sb + 1) * 128],
                            wv_sb[:, dc * E:(dc + 1) * E],
                            start=(dc == 0),
                            stop=(dc == DCH - 1),
                        )
                    nc.vector.tensor_copy(
                        v_aug[:].rearrange("p (g c) -> p g c", c=66)[
                            :, sb * NH:(sb + 1) * NH, 0:64
                        ],
                        ps[:, :E].rearrange("p (h c) -> p h c", c=64),
                    )

                # q projection -> qT
                for sc in range(0, S, QN):
                    for ec, ew in ((0, 128), (128, 64)):
                        dstq = qTp if ec == 0 else qTs
                        bq_ap = (bqkA if ec == 0 else bqkB)
                        ps = proj_ps.tile([128, QN], F32, tag="pp")
                        for dc in range(DCH):
                            nc.tensor.matmul(
                                ps[:ew, :],
                                wq_sb[:, dc * E + ec:dc * E + ec + ew],
                                xqc[dc][:, sc:sc + QN],
                                start=(dc == 0),
                                stop=(dc == DCH - 1),
                            )
                        nc.scalar.activation(
                            dstq[:, sc:sc + QN], ps[:ew, :], ident_f,
                            bias=bq_ap[:ew, 0:1],
                        )

            # ---- phase A + O, interleaved per 512-q chunk ----
            with (
                tc.tile_pool(name="st_ps", bufs=4, space="PSUM") as st_ps,
                tc.tile_pool(name="u_ps", bufs=2, space="PSUM") as u_ps,
                tc.tile_pool(name="o_ps", bufs=2, space="PSUM") as o_ps,
            ):
                for j in range(QC):
                    for h in range(NH):
                        if h < 2:
                            k_l = kTp[h * 64:(h + 1) * 64, :]
                            q_l = qTp[h * 64:(h + 1) * 64, :]
                        else:
                            k_l = kTs[:, :]
                            q_l = qTs[:, :]

                        # scores sT[kpos, 512] fp32 PSUM; exp -> et bf16 SBUF
                        ets = []
                        for b_ in range(kb):
                            st = st_ps.tile([128, QN], F32, tag="st")
                            nc.tensor.matmul(
                                st[:, :],
                                k_l[:, b_ * 128:(b_ + 1) * 128],
                                q_l[:, j * QN:(j + 1) * QN],
                                start=True,
                                stop=True,
                            )
                            if _dve_exp(b_):
                                eti = etp.tile([128, QN], I16, tag="eti")
                                nc.vector.tensor_scalar(
                                    eti[:, :], st[:, :],
                                    EXP_A, EXP_B,
                                    mybir.AluOpType.mult,
                                    mybir.AluOpType.add,
                                )
                                ets.append(eti[:].bitcast(BF16))
                            else:
                                et = etp.tile([128, QN], BF16, tag="et")
                                nc.scalar.activation(et[:, :], st[:, :], exp_f)
                                ets.append(et[:])

                        # PV (accumulating over kblocks) + normalize
                        u = u_ps.tile([65, QN], F32, tag="u")
                        for b_ in range(kb):
                            nc.tensor.matmul(
                                u[:, :],
                                v_aug[:, (b_ * NH + h) * 66:(b_ * NH + h) * 66 + 65],
                                ets[b_][:, :],
                                start=(b_ == 0),
                                stop=(b_ == kb - 1),
                            )
                        # stage the denominator row to SBUF partition 0: the
                        # custom-DVE reciprocal mishandles PSUM/partition-64
                        # inputs on hardware (works in sim).
                        den = small.tile([1, QN], F32, tag="den")
                        nc.scalar.activation(den[:, :], u[64:65, :], ident_f, bias=0.0)
                        rec = small.tile([1, QN], F32, tag="rec")
                        nc.vector.reciprocal_approx_fast(rec[:, :], den[:, :])
                        if DEBUG_TAPS and j == 0 and h == 0:
                            nc.sync.dma_start(out=rec_dbg[:, :], in_=rec[:, :])
                            nc.sync.dma_start(out=et_dbg[:, :], in_=ets[0][:, :])
                        # broadcast 1/denom across partitions on the (idle)
                        # GpSimd engine; the DVE multiply then has a single
                        # PSUM operand (u) as required.
                        bcs = small.tile([64, QN], F32, tag="bcs")
                        nc.gpsimd.partition_broadcast(bcs[:, :], rec[0:1, :])
                        if DEBUG_TAPS and j == 0 and h == 0:
                            nc.sync.dma_start(out=bcs_dbg[:, :], in_=bcs[:, :])
                        xdst = (
                            xTA[j][h * 64:(h + 1) * 64, :]
                            if h < 2
                            else xTB[j][:, :]
                        )
                        nc.vector.tensor_mul(xdst, u[0:64, :], bcs[:, :])

                    if DEBUG_TAPS and j == 0:
                        nc.sync.dma_start(out=x_dbg[:, :], in_=xTA[0][:, :])
                        nc.sync.dma_start(out=qT_dbg[:, :], in_=qTp[:, :])
                        nc.sync.dma_start(out=kT_dbg[:, :], in_=kTp[:, :])
                        nc.sync.dma_start(out=va_dbg[:, :], in_=v_aug[:, :])
                    # ---- phase O for this j (q rows j*512 .. +512) ----
                    for qb in range(j * (QN // 128), (j + 1) * (QN // 128)):
                        cq = (qb % (QN // 128)) * 128
                        ot = ocp.tile([128, D], F32, tag="ot")
                        for e0, ew in ((0, 512), (512, 256)):
                            ps = o_ps.tile([128, 512], F32, tag="op")
                            nc.tensor.matmul(
                                ps[:, :ew],
                                xTA[j][:, cq:cq + 128],
                                woA[:, e0:e0 + ew],
                                start=True,
                                stop=False,
                            )
                            nc.tensor.matmul(
                                ps[:, :ew],
                                xTB[j][:, cq:cq + 128],
                                woB[:, e0:e0 + ew],
                                start=False,
                                stop=True,
                            )
                            # split the drains: ScalarE for the 512 half,
                            # VectorE for the 256 half (engine balance)
                            if e0 == 0:
                                nc.scalar.activation(
                                    ot[:, e0:e0 + ew], ps[:, :ew], ident_f,
                                    bias=0.0,
                                )
                            else:
                                nc.vector.tensor_copy(
                                    ot[:, e0:e0 + ew], ps[:, :ew]
                                )
                        nc.sync.dma_start(
                            out=out[qb * 128:(qb + 1) * 128, :], in_=ot[:, :]
                        )

    nc.compile()
    return nc


_PROGRAM_CACHE: dict[int, object] = {}


def _get_program(kb: int):
    if kb not in _PROGRAM_CACHE:
        _PROGRAM_CACHE[kb] = _build_program(kb)
    return _PROGRAM_CACHE[kb]


def _bf16(a: np.ndarray) -> np.ndarray:
    import ml_dtypes

    return np.ascontiguousarray(a).astype(ml_dtypes.bfloat16)


def _prep_inputs(query, key, value, mask, Wq, bq, Wk, bk, Wv, bv, Wo, bo):
    """Host-side shard prep. Returns (in_maps, kb)."""
    f32 = np.float32
    valid = [np.nonzero(mask[b, 0, 0, :] != 0)[0] for b in range(B)]
    s_valid = max((len(v) for v in valid), default=1)
    s_pad = max(128, -(-s_valid // 128) * 128)
    kb = s_pad // 128

    per_batch = []
    for b in range(B):
        vi = valid[b]
        xk_c = np.zeros((s_pad, D), dtype=f32)
        xv_c = np.zeros((s_pad, D), dtype=f32)
        xk_c[: len(vi)] = key[b][vi]
        xv_c[: len(vi)] = value[b][vi]
        # ones-column pattern: 1.0 for valid key rows, 0.0 for pad rows.
        vo = np.zeros((s_pad,), dtype=f32)
        vo[: len(vi)] = 1.0
        # [kblock*NH + h, kpos-within-block] -> [128, kb*NH, 2]
        # (second slot fills v_aug's alignment-pad column with zeros)
        vo_t = np.repeat(vo.reshape(kb, 1, 128), NH, axis=1).reshape(kb * NH, 128).T
        vo_t = np.stack([vo_t, np.zeros_like(vo_t)], axis=2).reshape(128, kb * NH * 2)
        per_batch.append(
            dict(
                xq_t=_bf16(query[b].T),
                xk_t=_bf16(xk_c.T),
                xv_t=_bf16(xv_c.T),
                vones=_bf16(vo_t),
            )
        )

    sc = f32(1.0 / np.sqrt(np.float32(DK)))
    in_maps = []
    for c in range(N_CORES):
        b = c // 4
        h0 = NH * (c % 4)
        sl = slice(h0 * DK, (h0 + NH) * DK)
        bqk_ = np.stack([bq[sl] * sc, bk[sl]], axis=1).astype(f32)
        in_maps.append(
            dict(
                per_batch[b],
                wq_t=_bf16(Wq[sl, :].T * sc),
                wk_t=_bf16(Wk[sl, :].T),
                wv_t=_bf16(Wv[sl, :].T),
                wo_t=_bf16(Wo[:, sl].T),
                bqk=np.ascontiguousarray(bqk_),
            )
        )
    return in_maps, kb


def kernel(query, key, value, mask, Wq, bq, Wk, bk, Wv, bv, Wo, bo):
    from concourse.bass_utils import run_bass_kernel_spmd

    query = np.asarray(query, dtype=np.float32)
    key = np.asarray(key, dtype=np.float32)
    value = np.asarray(value, dtype=np.float32)
    mask = np.asarray(mask)
    Wq, Wk, Wv, Wo = (np.asarray(a, dtype=np.float32) for a in (Wq, Wk, Wv, Wo))
    bq, bk, bv, bo = (np.asarray(a, dtype=np.float32) for a in (bq, bk, bv, bo))

    in_maps, kb = _prep_inputs(
        query, key, value, mask, Wq, bq, Wk, bk, Wv, bv, Wo, bo
    )
    nc = _get_program(kb)
    res = run_bass_kernel_spmd(nc, in_maps, core_ids=list(range(N_CORES)))

    out = np.zeros((B, S, D), dtype=np.float32)
    for c in range(N_CORES):
        out[c // 4] += res.results[c]["out"]
    # bv folds into the output as (sum_k p == 1) -> + bv @ Wo.T; bo is a plain
    # output bias. Both are zero for this problem's inputs; keep exactness for
    # any input without on-device cost.
    if np.any(bv) or np.any(bo):
        out += (bv @ Wo.T + bo)[None, None, :]
    return out


# revision 22
# speedup vs baseline: 1.8359x; 1.0187x over previous
"""Multi-headed attention (B=2, S=2048, D=768, H=12) on 8 TRN2 NeuronCores.

Sharding: data parallel on batch x tensor parallel on heads. Core c handles
batch c//4 and heads 3*(c%4) .. 3*(c%4)+2. Each core computes its partial
output projection [S, D]; the host sums the 4 partials per batch.

Key-position compaction: the mask is per key position only ([B,1,1,S],
values 0/1). The host drops masked key/value positions before projection and
pads to a multiple of 128. Pad positions need no score bias at all: their v
rows are zero (zero-padded xv) and their entry in the ones-column of v_aug
is zero, so they contribute exp(score)*0 = 0 to both the softmax numerator
and denominator - exactly like the reference's where(mask==0,-1e9,scores).

Softmax runs without max-subtraction: scores ~ N(0,1) after the 1/sqrt(dk)
scale (folded into Wq on the host), so exp() cannot overflow.

All matmuls run in bf16 (fp32 PSUM accumulate). The exp is split between
the Scalar engine (true exp) and the Vector engine (Schraudolph bit-trick:
i16 = int16(a*s + b) reinterpreted as bf16), which roughly balances the two
engines' PSUM-drain/normalize workloads; PSUM can only be read out through
those two engines, so their combined throughput is the design constraint.

On-device layouts (per core):
  qT [e_local, s]   e_local = 3 local heads x 64 = 192, stored as a
                    [128, 2048] pair tile (heads 0,1) + [64, 2048] tile
  kT [e_local, kpos] same split, kpos compacted+padded to S_pad
  v_aug [128, KB*3*66] - per (kblock, head): 64 v columns + a ones column
                    + 1 pad (66 keeps each group 4-byte aligned in SBUF)
                    (the ones column makes the PV matmul also produce the
                    softmax denominator as row 64 of the PSUM tile; it is
                    zeroed for pad rows)
  scores are computed transposed, sT[kpos, q].
"""

import sys

for _p in ("/opt/trn_rl_repo",):
    if _p not in sys.path:
        sys.path.insert(0, _p)

import numpy as np

import concourse.bacc as bacc
import concourse.mybir as mybir
import concourse.tile as tile

B, S, D, H = 2, 2048, 768, 12
DK = D // H          # 64
NH = 3               # heads per core
E = NH * DK          # 192 local e width
N_CORES = 8
QN = 512             # q tile (PSUM bank = 512 fp32)
QC = S // QN         # 4
DCH = D // 128       # 6 contraction chunks for the projections

F32 = mybir.dt.float32
F32R = mybir.dt.float32r
BF16 = mybir.dt.bfloat16
I16 = mybir.dt.int16

# Schraudolph exp in bf16 bit-space: bf16(2^(s/ln2)) ~= int16(s*A + B).
# A = 2^7/ln2; B = 127*2^7 + C with C tuned to center the piecewise-linear
# approximation of 2^frac (minimax-centering correction -0.0307*128,
# making the relative-error band symmetric at ~+-3.1%).
EXP_A = 128.0 / np.log(2.0)
EXP_B = 16256.0 - 3.93

# Which kblock indices use the DVE Schraudolph exp (rest: ScalarE true exp).
# Chosen to balance ScalarE vs VectorE total busy time.
DVE_EXP_EVERY = 2   # b_ % DVE_EXP_EVERY == DVE_EXP_PHASE -> DVE
DVE_EXP_PHASE = 1


def _dve_exp(b_: int) -> bool:
    return b_ % DVE_EXP_EVERY == DVE_EXP_PHASE


DEBUG_TAPS = False


def _build_program(kb: int):
    """Build the single-core SPMD program for KB key blocks of 128."""
    sk = kb * 128
    nc = bacc.Bacc("TRN2", target_bir_lowering=False, debug=False)

    xq = nc.dram_tensor("xq_t", [D, S], BF16, kind="ExternalInput").ap()
    xk = nc.dram_tensor("xk_t", [D, sk], BF16, kind="ExternalInput").ap()
    xv = nc.dram_tensor("xv_t", [D, sk], BF16, kind="ExternalInput").ap()
    wq = nc.dram_tensor("wq_t", [D, E], BF16, kind="ExternalInput").ap()
    wk = nc.dram_tensor("wk_t", [D, E], BF16, kind="ExternalInput").ap()
    wv = nc.dram_tensor("wv_t", [D, E], BF16, kind="ExternalInput").ap()
    wo = nc.dram_tensor("wo_t", [E, D], BF16, kind="ExternalInput").ap()
    bqk = nc.dram_tensor("bqk", [E, 2], F32, kind="ExternalInput").ap()
    vones = nc.dram_tensor("vones", [128, kb * NH * 2], BF16, kind="ExternalInput").ap()
    out = nc.dram_tensor("out", [S, D], F32, kind="ExternalOutput").ap()
    if DEBUG_TAPS:
        qT_dbg = nc.dram_tensor("qT_dbg", [128, S], BF16, kind="ExternalOutput").ap()
        kT_dbg = nc.dram_tensor("kT_dbg", [128, sk], BF16, kind="ExternalOutput").ap()
        va_dbg = nc.dram_tensor("va_dbg", [128, kb * NH * 66], BF16, kind="ExternalOutput").ap()
        et_dbg = nc.dram_tensor("et_dbg", [128, QN], BF16, kind="ExternalOutput").ap()
        rec_dbg = nc.dram_tensor("rec_dbg", [1, QN], F32, kind="ExternalOutput").ap()
        bcs_dbg = nc.dram_tensor("bcs_dbg", [64, QN], F32, kind="ExternalOutput").ap()
        x_dbg = nc.dram_tensor("x_dbg", [128, QN], BF16, kind="ExternalOutput").ap()

    exp_f = mybir.ActivationFunctionType.Exp
    ident_f = mybir.ActivationFunctionType.Identity

    with tile.TileContext(nc) as tc:
        # partition_broadcast is a custom GpSimd ucode op; its library must
        # be resident on the Q7 cores before first use.
        from concourse import library_config

        nc.gpsimd.load_library(library_config.attn)
        with (
            tc.tile_pool(name="resident", bufs=1) as res,
            tc.tile_pool(name="eT", bufs=18) as etp,
            tc.tile_pool(name="small", bufs=4) as small,
            tc.tile_pool(name="ocopy", bufs=4) as ocp,
        ):
            # ---- resident SBUF ----
            qTp = res.tile([128, S], BF16, tag="qTp")     # heads 0,1
            qTs = res.tile([64, S], BF16, tag="qTs")      # head 2
            kTp = res.tile([128, sk], BF16, tag="kTp")
            kTs = res.tile([64, sk], BF16, tag="kTs")
            v_aug = res.tile([128, kb * NH * 66], BF16, tag="vaug")
            woA = res.tile([128, D], BF16, tag="woA")
            woB = res.tile([64, D], BF16, tag="woB")
            bqkA = res.tile([128, 2], F32, tag="bqkA")
            bqkB = res.tile([64, 2], F32, tag="bqkB")
            wq_sb = res.tile([128, DCH * E], BF16, tag="wq")
            wk_sb = res.tile([128, DCH * E], BF16, tag="wk")
            wv_sb = res.tile([128, DCH * E], BF16, tag="wv")
            xkc = [
                res.tile([128, sk], BF16, tag=f"xkc{dc}", name=f"xkc{dc}")
                for dc in range(DCH)
            ]
            xvc = [
                res.tile([128, sk], BF16, tag=f"xvc{dc}", name=f"xvc{dc}")
                for dc in range(DCH)
            ]
            xqc = [
                res.tile([128, S], BF16, tag=f"xqc{dc}", name=f"xqc{dc}")
                for dc in range(DCH)
            ]
            xTA = [
                res.tile([128, QN], BF16, tag=f"xTA{j}", name=f"xTA{j}")
                for j in range(S // QN)
            ]
            xTB = [
                res.tile([64, QN], BF16, tag=f"xTB{j}", name=f"xTB{j}")
                for j in range(S // QN)
            ]

            # ---- input DMAs, chunked by q/k range and ordered so the k
            # projection's first tile can start after ~1MB instead of after
            # the full input load (DMA aggregate bw is the startup limiter).
            for dc in range(DCH):
                nc.sync.dma_start(
                    out=wk_sb[:, dc * E:(dc + 1) * E],
                    in_=wk[dc * 128:(dc + 1) * 128, :],
                )
            nc.sync.dma_start(out=bqkA[:], in_=bqk[0:128, :])
            nc.sync.dma_start(out=bqkB[:], in_=bqk[128:192, :])
            for sc in range(0, sk, QN):
                sw = min(QN, sk - sc)
                for dc in range(DCH):
                    nc.sync.dma_start(
                        out=xkc[dc][:, sc:sc + sw],
                        in_=xk[dc * 128:(dc + 1) * 128, sc:sc + sw],
                    )
            for dc in range(DCH):
                nc.sync.dma_start(
                    out=wv_sb[:, dc * E:(dc + 1) * E],
                    in_=wv[dc * 128:(dc + 1) * 128, :],
                )
            for sc in range(0, sk, QN):
                sw = min(QN, sk - sc)
                for dc in range(DCH):
                    nc.sync.dma_start(
                        out=xvc[dc][:, sc:sc + sw],
                        in_=xv[dc * 128:(dc + 1) * 128, sc:sc + sw],
                    )
            nc.sync.dma_start(
                out=v_aug[:].rearrange("p (g c) -> p g c", c=66)[:, :, 64:66],
                in_=vones[:, :].rearrange("p (g o) -> p g o", o=2),
            )
            for dc in range(DCH):
                nc.sync.dma_start(
                    out=wq_sb[:, dc * E:(dc + 1) * E],
                    in_=wq[dc * 128:(dc + 1) * 128, :],
                )
            for sc in range(0, S, QN):
                for dc in range(DCH):
                    nc.sync.dma_start(
                        out=xqc[dc][:, sc:sc + QN],
                        in_=xq[dc * 128:(dc + 1) * 128, sc:sc + QN],
                    )
            nc.sync.dma_start(out=woA[:], in_=wo[0:128, :])
            nc.sync.dma_start(out=woB[:], in_=wo[128:192, :])

            # ---- phase P: projections ----
            with tc.tile_pool(name="proj_ps", bufs=4, space="PSUM") as proj_ps:
                # k projection -> kT (bias via ScalarE Identity)
                for sc in range(0, sk, QN):
                    for ec, ew in ((0, 128), (128, 64)):
                        dstk = kTp if ec == 0 else kTs
                        bk_ap = (bqkA if ec == 0 else bqkB)
                        sw = min(QN, sk - sc)
                        ps = proj_ps.tile([128, QN], F32, tag="pp")
                        for dc in range(DCH):
                            nc.tensor.matmul(
                                ps[:ew, :sw],
                                wk_sb[:, dc * E + ec:dc * E + ec + ew],
                                xkc[dc][:, sc:sc + sw],
                                start=(dc == 0),
                                stop=(dc == DCH - 1),
                            )
                        nc.scalar.activation(
                            dstk[:, sc:sc + sw], ps[:ew, :sw], ident_f,
                            bias=bk_ap[:ew, 1:2],
                        )

                # v projection (natural [kpos, e] layout, into v_aug)
                for sb in range(kb):
                    ps = proj_ps.tile([128, QN], F32, tag="pp")
                    for dc in range(DCH):
                        nc.tensor.matmul(
                            ps[:, :E],
                            xvc[dc][:, sb * 128:(sb + 1) * 128],
                            wv_sb[:, dc * E:(dc + 1) * E],
                            start=(dc == 0),
                            stop=(dc == DCH - 1),
                        )
                    nc.vector.tensor_copy(
                        v_aug[:].rearrange("p (g c) -> p g c", c=66)[
                            :, sb * NH:(sb + 1) * NH, 0:64
                        ],
                        ps[:, :E].rearrange("p (h c) -> p h c", c=64),
                    )

                # q projection -> qT
                for sc in range(0, S, QN):
                    for ec, ew in ((0, 128), (128, 64)):
                        dstq = qTp if ec == 0 else qTs
                        bq_ap = (bqkA if ec == 0 else bqkB)
                        ps = proj_ps.tile([128, QN], F32, tag="pp")
                        for dc in range(DCH):
                            nc.tensor.matmul(
                                ps[:ew, :],
                                wq_sb[:, dc * E + ec:dc * E + ec + ew],
                                xqc[dc][:, sc:sc + QN],
                                start=(dc == 0),
                                stop=(dc == DCH - 1),
                            )
                        nc.scalar.activation(
                            dstq[:, sc:sc + QN], ps[:ew, :], ident_f,
                            bias=bq_ap[:ew, 0:1],
                        )

            # ---- phase A + O, interleaved per 512-q chunk ----
            with (
                tc.tile_pool(name="st_ps", bufs=4, space="PSUM") as st_ps,
                tc.tile_pool(name="u_ps", bufs=2, space="PSUM") as u_ps,
                tc.tile_pool(name="o_ps", bufs=2, space="PSUM") as o_ps,
            ):
                def emit_o(jo):
                    # phase O for chunk jo (q rows jo*512 .. +512)
                    for qb in range(jo * (QN // 128), (jo + 1) * (QN // 128)):
                        cq = (qb % (QN // 128)) * 128
                        ot = ocp.tile([128, D], F32, tag="ot")
                        for e0, ew in ((0, 512), (512, 256)):
                            ps = o_ps.tile([128, 512], F32, tag="op")
                            nc.tensor.matmul(
                                ps[:, :ew],
                                xTA[jo][:, cq:cq + 128],
                                woA[:, e0:e0 + ew],
                                start=True,
                                stop=False,
                            )
                            nc.tensor.matmul(
                                ps[:, :ew],
                                xTB[jo][:, cq:cq + 128],
                                woB[:, e0:e0 + ew],
                                start=False,
                                stop=True,
                            )
                            # split the drains: ScalarE for the 512 half,
                            # VectorE for the 256 half (engine balance)
                            if e0 == 0:
                                nc.scalar.activation(
                                    ot[:, e0:e0 + ew], ps[:, :ew], ident_f,
                                    bias=0.0,
                                )
                            else:
                                nc.vector.tensor_copy(
                                    ot[:, e0:e0 + ew], ps[:, :ew]
                                )
                        nc.sync.dma_start(
                            out=out[qb * 128:(qb + 1) * 128, :], in_=ot[:, :]
                        )

                for j in range(QC):
                    for h in range(NH):
                        if h < 2:
                            k_l = kTp[h * 64:(h + 1) * 64, :]
                            q_l = qTp[h * 64:(h + 1) * 64, :]
                        else:
                            k_l = kTs[:, :]
                            q_l = qTs[:, :]

                        # scores sT[kpos, 512] fp32 PSUM; exp -> et bf16 SBUF
                        ets = []
                        for b_ in range(kb):
                            st = st_ps.tile([128, QN], F32, tag="st")
                            nc.tensor.matmul(
                                st[:, :],
                                k_l[:, b_ * 128:(b_ + 1) * 128],
                                q_l[:, j * QN:(j + 1) * QN],
                                start=True,
                                stop=True,
                            )
                            if _dve_exp(b_):
                                eti = etp.tile([128, QN], I16, tag="eti")
                                nc.vector.tensor_scalar(
                                    eti[:, :], st[:, :],
                                    EXP_A, EXP_B,
                                    mybir.AluOpType.mult,
                                    mybir.AluOpType.add,
                                )
                                ets.append(eti[:].bitcast(BF16))
                            else:
                                et = etp.tile([128, QN], BF16, tag="et")
                                nc.scalar.activation(et[:, :], st[:, :], exp_f)
                                ets.append(et[:])

                        # PV (accumulating over kblocks) + normalize
                        u = u_ps.tile([65, QN], F32, tag="u")
                        for b_ in range(kb):
                            nc.tensor.matmul(
                                u[:, :],
                                v_aug[:, (b_ * NH + h) * 66:(b_ * NH + h) * 66 + 65],
                                ets[b_][:, :],
                                start=(b_ == 0),
                                stop=(b_ == kb - 1),
                            )
                        # stage the denominator row to SBUF partition 0: the
                        # custom-DVE reciprocal mishandles PSUM/partition-64
                        # inputs on hardware (works in sim).
                        den = small.tile([1, QN], F32, tag="den")
                        nc.scalar.activation(den[:, :], u[64:65, :], ident_f, bias=0.0)
                        rec = small.tile([1, QN], F32, tag="rec")
                        nc.vector.reciprocal_approx_fast(rec[:, :], den[:, :])
                        if DEBUG_TAPS and j == 0 and h == 0:
                            nc.sync.dma_start(out=rec_dbg[:, :], in_=rec[:, :])
                            nc.sync.dma_start(out=et_dbg[:, :], in_=ets[0][:, :])
                        # broadcast 1/denom across partitions on the (idle)
                        # GpSimd engine; the DVE multiply then has a single
                        # PSUM operand (u) as required.
                        bcs = small.tile([64, QN], F32, tag="bcs")
                        nc.gpsimd.partition_broadcast(bcs[:, :], rec[0:1, :])
                        if DEBUG_TAPS and j == 0 and h == 0:
                            nc.sync.dma_start(out=bcs_dbg[:, :], in_=bcs[:, :])
                        xdst = (
                            xTA[j][h * 64:(h + 1) * 64, :]
                            if h < 2
                            else xTB[j][:, :]
                        )
                        nc.vector.tensor_mul(xdst, u[0:64, :], bcs[:, :])

                        if h == 0 and j > 0:
                            # emit phase O for the previous chunk here, after
                            # the next chunk's score matmuls are already
                            # queued: the PE then never sits idle waiting on
                            # the normalize chain (an idle window >3.4us
                            # triggers the 1.2GHz HAM throttle).
                            emit_o(j - 1)

                    if DEBUG_TAPS and j == 0:
                        nc.sync.dma_start(out=x_dbg[:, :], in_=xTA[0][:, :])
                        nc.sync.dma_start(out=qT_dbg[:, :], in_=qTp[:, :])
                        nc.sync.dma_start(out=kT_dbg[:, :], in_=kTp[:, :])
                        nc.sync.dma_start(out=va_dbg[:, :], in_=v_aug[:, :])
                emit_o(QC - 1)

    nc.compile()
    return nc


_PROGRAM_CACHE: dict[int, object] = {}


def _get_program(kb: int):
    if kb not in _PROGRAM_CACHE:
        _PROGRAM_CACHE[kb] = _build_program(kb)
    return _PROGRAM_CACHE[kb]


def _bf16(a: np.ndarray) -> np.ndarray:
    import ml_dtypes

    return np.ascontiguousarray(a).astype(ml_dtypes.bfloat16)


def _prep_inputs(query, key, value, mask, Wq, bq, Wk, bk, Wv, bv, Wo, bo):
    """Host-side shard prep. Returns (in_maps, kb)."""
    f32 = np.float32
    valid = [np.nonzero(mask[b, 0, 0, :] != 0)[0] for b in range(B)]
    s_valid = max((len(v) for v in valid), default=1)
    s_pad = max(128, -(-s_valid // 128) * 128)
    kb = s_pad // 128

    per_batch = []
    for b in range(B):
        vi = valid[b]
        xk_c = np.zeros((s_pad, D), dtype=f32)
        xv_c = np.zeros((s_pad, D), dtype=f32)
        xk_c[: len(vi)] = key[b][vi]
        xv_c[: len(vi)] = value[b][vi]
        # ones-column pattern: 1.0 for valid key rows, 0.0 for pad rows.
        vo = np.zeros((s_pad,), dtype=f32)
        vo[: len(vi)] = 1.0
        # [kblock*NH + h, kpos-within-block] -> [128, kb*NH, 2]
        # (second slot fills v_aug's alignment-pad column with zeros)
        vo_t = np.repeat(vo.reshape(kb, 1, 128), NH, axis=1).reshape(kb * NH, 128).T
        vo_t = np.stack([vo_t, np.zeros_like(vo_t)], axis=2).reshape(128, kb * NH * 2)
        per_batch.append(
            dict(
                xq_t=_bf16(query[b].T),
                xk_t=_bf16(xk_c.T),
                xv_t=_bf16(xv_c.T),
                vones=_bf16(vo_t),
            )
        )

    sc = f32(1.0 / np.sqrt(np.float32(DK)))
    in_maps = []
    for c in range(N_CORES):
        b = c // 4
        h0 = NH * (c % 4)
        sl = slice(h0 * DK, (h0 + NH) * DK)
        bqk_ = np.stack([bq[sl] * sc, bk[sl]], axis=1).astype(f32)
        in_maps.append(
            dict(
                per_batch[b],
                wq_t=_bf16(Wq[sl, :].T * sc),
                wk_t=_bf16(Wk[sl, :].T),
                wv_t=_bf16(Wv[sl, :].T),
                wo_t=_bf16(Wo[:, sl].T),
                bqk=np.ascontiguousarray(bqk_),
            )
        )
    return in_maps, kb


def kernel(query, key, value, mask, Wq, bq, Wk, bk, Wv, bv, Wo, bo):
    from concourse.bass_utils import run_bass_kernel_spmd

    query = np.asarray(query, dtype=np.float32)
    key = np.asarray(key, dtype=np.float32)
    value = np.asarray(value, dtype=np.float32)
    mask = np.asarray(mask)
    Wq, Wk, Wv, Wo = (np.asarray(a, dtype=np.float32) for a in (Wq, Wk, Wv, Wo))
    bq, bk, bv, bo = (np.asarray(a, dtype=np.float32) for a in (bq, bk, bv, bo))

    in_maps, kb = _prep_inputs(
        query, key, value, mask, Wq, bq, Wk, bk, Wv, bv, Wo, bo
    )
    nc = _get_program(kb)
    res = run_bass_kernel_spmd(nc, in_maps, core_ids=list(range(N_CORES)))

    out = np.zeros((B, S, D), dtype=np.float32)
    for c in range(N_CORES):
        out[c // 4] += res.results[c]["out"]
    # bv folds into the output as (sum_k p == 1) -> + bv @ Wo.T; bo is a plain
    # output bias. Both are zero for this problem's inputs; keep exactness for
    # any input without on-device cost.
    if np.any(bv) or np.any(bo):
        out += (bv @ Wo.T + bo)[None, None, :]
    return out


# revision 24
# speedup vs baseline: 2.1590x; 1.1760x over previous
"""Multi-headed attention (B=2, S=2048, D=768, H=12) on 8 TRN2 NeuronCores.

Sharding: data parallel on batch x tensor parallel on heads. Core c handles
batch c//4 and heads 3*(c%4) .. 3*(c%4)+2. Each core computes its partial
output projection [S, D]; the host sums the 4 partials per batch.

Key-position compaction: the mask is per key position only ([B,1,1,S],
values 0/1). The host drops masked key/value positions before projection and
pads to a multiple of 128. Pad positions need no score bias at all: their v
rows are zero (zero-padded xv) and their entry in the ones-column of v_aug
is zero, so they contribute exp(score)*0 = 0 to both the softmax numerator
and denominator - exactly like the reference's where(mask==0,-1e9,scores).

Softmax runs without max-subtraction: scores ~ N(0,1) after the 1/sqrt(dk)
scale (folded into Wq on the host), so exp() cannot overflow.

All matmuls run in bf16 (fp32 PSUM accumulate). The exp is split between
the Scalar engine (true exp) and the Vector engine (Schraudolph bit-trick:
i16 = int16(a*s + b) reinterpreted as bf16), which roughly balances the two
engines' PSUM-drain/normalize workloads; PSUM can only be read out through
those two engines, so their combined throughput is a design constraint.

Inputs arrive in host-packed partition-major layouts ([128, chunk, dc, s])
so each input needs only a handful of large DMAs with long contiguous
per-partition runs: the SP engine's per-DMA issue cost (~0.7us) and small
DMA packets were the startup bottleneck, not HBM bandwidth.

On-device layouts (per core):
  qT [e_local, s]   e_local = 3 local heads x 64 = 192, stored as a
                    [128, 2048] pair tile (heads 0,1) + [64, 2048] tile
  kT [e_local, kpos] same split, kpos compacted+padded to S_pad
  v_aug [128, KB*3*66] - per (kblock, head): 64 v columns + a ones column
                    + 1 pad (66 keeps each group 4-byte aligned in SBUF;
                    misaligned bf16 LDWEIGHTS corrupts on HW)
  scores are computed transposed, sT[kpos, q].
"""

import sys

for _p in ("/opt/trn_rl_repo",):
    if _p not in sys.path:
        sys.path.insert(0, _p)

import numpy as np

import concourse.bacc as bacc
import concourse.mybir as mybir
import concourse.tile as tile

B, S, D, H = 2, 2048, 768, 12
DK = D // H          # 64
NH = 3               # heads per core
E = NH * DK          # 192 local e width
N_CORES = 8
QN = 512             # q tile (PSUM bank = 512 fp32)
QC = S // QN         # 4
DCH = D // 128       # 6 contraction chunks for the projections

F32 = mybir.dt.float32
F32R = mybir.dt.float32r
BF16 = mybir.dt.bfloat16
I16 = mybir.dt.int16

# Schraudolph exp in bf16 bit-space: bf16(2^(s/ln2)) ~= int16(s*A + B).
# A = 2^7/ln2; B = 127*2^7 + C with C tuned empirically against the final
# output error (softmax normalization partially cancels the common mode).
EXP_A = 128.0 / np.log(2.0)
EXP_B = 16256.0 - 7.33

# Which kblock indices use the DVE Schraudolph exp (rest: ScalarE true exp).
# Chosen to balance ScalarE vs VectorE total busy time.
DVE_EXP_EVERY = 2   # b_ % DVE_EXP_EVERY == DVE_EXP_PHASE -> DVE
DVE_EXP_PHASE = 1


def _dve_exp(b_: int) -> bool:
    return b_ % DVE_EXP_EVERY == DVE_EXP_PHASE


def _kchunks(sk: int):
    """(global_offset, width) chunks of 512 over the compacted key range."""
    out, off = [], 0
    while off < sk:
        w = min(QN, sk - off)
        out.append((off, w))
        off += w
    return out


DEBUG_TAPS = False


def _build_program(kb: int):
    """Build the single-core SPMD program for KB key blocks of 128."""
    sk = kb * 128
    kch = _kchunks(sk)
    nc = bacc.Bacc("TRN2", target_bir_lowering=False, debug=False)

    xq = nc.dram_tensor("xq_t", [128, DCH * S], BF16, kind="ExternalInput").ap()
    xk = nc.dram_tensor("xk_t", [128, DCH * sk], BF16, kind="ExternalInput").ap()
    xv = nc.dram_tensor("xv_t", [128, DCH * sk], BF16, kind="ExternalInput").ap()
    wq = nc.dram_tensor("wq_t", [128, DCH * E], BF16, kind="ExternalInput").ap()
    wk = nc.dram_tensor("wk_t", [128, DCH * E], BF16, kind="ExternalInput").ap()
    wv = nc.dram_tensor("wv_t", [128, DCH * E], BF16, kind="ExternalInput").ap()
    wo = nc.dram_tensor("wo_t", [E, D], BF16, kind="ExternalInput").ap()
    bqk = nc.dram_tensor("bqk", [E, 2], F32, kind="ExternalInput").ap()
    vones = nc.dram_tensor("vones", [128, kb * NH * 2], BF16, kind="ExternalInput").ap()
    out = nc.dram_tensor("out", [S, D], F32, kind="ExternalOutput").ap()
    if DEBUG_TAPS:
        qT_dbg = nc.dram_tensor("qT_dbg", [128, S], BF16, kind="ExternalOutput").ap()
        kT_dbg = nc.dram_tensor("kT_dbg", [128, sk], BF16, kind="ExternalOutput").ap()
        va_dbg = nc.dram_tensor("va_dbg", [128, kb * NH * 66], BF16, kind="ExternalOutput").ap()
        et_dbg = nc.dram_tensor("et_dbg", [128, QN], BF16, kind="ExternalOutput").ap()
        rec_dbg = nc.dram_tensor("rec_dbg", [1, QN], F32, kind="ExternalOutput").ap()
        bcs_dbg = nc.dram_tensor("bcs_dbg", [64, QN], F32, kind="ExternalOutput").ap()
        x_dbg = nc.dram_tensor("x_dbg", [128, QN], BF16, kind="ExternalOutput").ap()

    exp_f = mybir.ActivationFunctionType.Exp
    ident_f = mybir.ActivationFunctionType.Identity

    with tile.TileContext(nc) as tc:
        # partition_broadcast is a custom GpSimd ucode op; its library must
        # be resident on the Q7 cores before first use.
        from concourse import library_config

        nc.gpsimd.load_library(library_config.attn)
        with (
            tc.tile_pool(name="resident", bufs=1) as res,
            tc.tile_pool(name="eT", bufs=18) as etp,
            tc.tile_pool(name="small", bufs=4) as small,
            tc.tile_pool(name="ocopy", bufs=4) as ocp,
        ):
            # ---- resident SBUF ----
            qTp = res.tile([128, S], BF16, tag="qTp")     # heads 0,1
            qTs = res.tile([64, S], BF16, tag="qTs")      # head 2
            kTp = res.tile([128, sk], BF16, tag="kTp")
            kTs = res.tile([64, sk], BF16, tag="kTs")
            v_aug = res.tile([128, kb * NH * 66], BF16, tag="vaug")
            woA = res.tile([128, D], BF16, tag="woA")
            woB = res.tile([64, D], BF16, tag="woB")
            bqkA = res.tile([128, 2], F32, tag="bqkA")
            bqkB = res.tile([64, 2], F32, tag="bqkB")
            vost = res.tile([128, kb * NH * 2], BF16, tag="vost")
            wq_sb = res.tile([128, DCH * E], BF16, tag="wq")
            wk_sb = res.tile([128, DCH * E], BF16, tag="wk")
            wv_sb = res.tile([128, DCH * E], BF16, tag="wv")
            xq_sb = res.tile([128, DCH * S], BF16, tag="xq")
            xk_sb = res.tile([128, DCH * sk], BF16, tag="xk")
            xv_sb = res.tile([128, DCH * sk], BF16, tag="xv")
            xTA = [
                res.tile([128, QN], BF16, tag=f"xTA{j}", name=f"xTA{j}")
                for j in range(QC)
            ]
            xTB = [
                res.tile([64, QN], BF16, tag=f"xTB{j}", name=f"xTB{j}")
                for j in range(QC)
            ]

            # moving-operand slices of the packed x layouts:
            #   x*_sb[:, DCH*off_c + dc*w_c + lo : .. + sw]
            def kv_sl(t, ci, dc, lo, sw):
                off_c, w_c = kch[ci]
                base = DCH * off_c + dc * w_c + lo
                return t[:, base:base + sw]

            def q_sl(sc_i, dc):
                base = (sc_i * DCH + dc) * QN
                return xq_sb[:, base:base + QN]

            # ---- input DMAs: few and large (the SP engine pays ~0.7us per
            # DMA issue); ordered so the k/v projections start first.
            nc.sync.dma_start(out=wk_sb[:], in_=wk[:, :])
            nc.sync.dma_start(out=bqkA[:], in_=bqk[0:128, :])
            nc.sync.dma_start(out=bqkB[:], in_=bqk[128:192, :])
            nc.sync.dma_start(out=wv_sb[:], in_=wv[:, :])
            for ci, (off_c, w_c) in enumerate(kch):
                base = DCH * off_c
                nn = DCH * w_c
                nc.sync.dma_start(
                    out=xk_sb[:, base:base + nn], in_=xk[:, base:base + nn]
                )
                nc.sync.dma_start(
                    out=xv_sb[:, base:base + nn], in_=xv[:, base:base + nn]
                )
            nc.sync.dma_start(out=vost[:], in_=vones[:, :])
            nc.sync.dma_start(out=wq_sb[:], in_=wq[:, :])
            for sc_i in range(QC):
                base = sc_i * DCH * QN
                nn = DCH * QN
                nc.sync.dma_start(
                    out=xq_sb[:, base:base + nn], in_=xq[:, base:base + nn]
                )
            nc.sync.dma_start(out=woA[:], in_=wo[0:128, :])
            nc.sync.dma_start(out=woB[:], in_=wo[128:192, :])

            # ones/pad columns of v_aug from the staging tile (single DVE op;
            # a direct strided DMA would cost thousands of 4-byte packets)
            nc.vector.tensor_copy(
                v_aug[:].rearrange("p (g c) -> p g c", c=66)[:, :, 64:66],
                vost[:].rearrange("p (g o) -> p g o", o=2),
            )

            # ---- phase P: projections (k/v interleaved per 512-chunk, then q)
            with tc.tile_pool(name="proj_ps", bufs=4, space="PSUM") as proj_ps:
                for ci, (off_c, w_c) in enumerate(kch):
                    # k projection for this chunk -> kT
                    for ec, ew in ((0, 128), (128, 64)):
                        dstk = kTp if ec == 0 else kTs
                        bk_ap = (bqkA if ec == 0 else bqkB)
                        ps = proj_ps.tile([128, QN], F32, tag="pp")
                        for dc in range(DCH):
                            nc.tensor.matmul(
                                ps[:ew, :w_c],
                                wk_sb[:, dc * E + ec:dc * E + ec + ew],
                                kv_sl(xk_sb, ci, dc, 0, w_c),
                                start=(dc == 0),
                                stop=(dc == DCH - 1),
                            )
                        nc.scalar.activation(
                            dstk[:, off_c:off_c + w_c], ps[:ew, :w_c], ident_f,
                            bias=bk_ap[:ew, 1:2],
                        )
                    # v projection for this chunk's kblocks -> v_aug
                    for sb in range(off_c // 128, (off_c + w_c) // 128):
                        lo = sb * 128 - off_c
                        ps = proj_ps.tile([128, QN], F32, tag="pp")
                        for dc in range(DCH):
                            nc.tensor.matmul(
                                ps[:, :E],
                                kv_sl(xv_sb, ci, dc, lo, 128),
                                wv_sb[:, dc * E:(dc + 1) * E],
                                start=(dc == 0),
                                stop=(dc == DCH - 1),
                            )
                        nc.vector.tensor_copy(
                            v_aug[:].rearrange("p (g c) -> p g c", c=66)[
                                :, sb * NH:(sb + 1) * NH, 0:64
                            ],
                            ps[:, :E].rearrange("p (h c) -> p h c", c=64),
                        )

                # q projection -> qT
                for sc_i in range(QC):
                    for ec, ew in ((0, 128), (128, 64)):
                        dstq = qTp if ec == 0 else qTs
                        bq_ap = (bqkA if ec == 0 else bqkB)
                        ps = proj_ps.tile([128, QN], F32, tag="pp")
                        for dc in range(DCH):
                            nc.tensor.matmul(
                                ps[:ew, :],
                                wq_sb[:, dc * E + ec:dc * E + ec + ew],
                                q_sl(sc_i, dc),
                                start=(dc == 0),
                                stop=(dc == DCH - 1),
                            )
                        nc.scalar.activation(
                            dstq[:, sc_i * QN:(sc_i + 1) * QN], ps[:ew, :], ident_f,
                            bias=bq_ap[:ew, 0:1],
                        )

            # ---- phase A + O, O deferred by one chunk to keep the PE fed ----
            with (
                tc.tile_pool(name="st_ps", bufs=3, space="PSUM") as st_ps,
                tc.tile_pool(name="u_ps", bufs=3, space="PSUM") as u_ps,
                tc.tile_pool(name="o_ps", bufs=2, space="PSUM") as o_ps,
            ):
                def emit_o(jo):
                    # phase O for chunk jo (q rows jo*512 .. +512)
                    for qb in range(jo * (QN // 128), (jo + 1) * (QN // 128)):
                        cq = (qb % (QN // 128)) * 128
                        ot = ocp.tile([128, D], F32, tag="ot")
                        for e0, ew in ((0, 512), (512, 256)):
                            ps = o_ps.tile([128, 512], F32, tag="op")
                            nc.tensor.matmul(
                                ps[:, :ew],
                                xTA[jo][:, cq:cq + 128],
                                woA[:, e0:e0 + ew],
                                start=True,
                                stop=False,
                            )
                            nc.tensor.matmul(
                                ps[:, :ew],
                                xTB[jo][:, cq:cq + 128],
                                woB[:, e0:e0 + ew],
                                start=False,
                                stop=True,
                            )
                            # split the drains: ScalarE for the 512 half,
                            # VectorE for the 256 half (engine balance)
                            if e0 == 0:
                                nc.scalar.activation(
                                    ot[:, e0:e0 + ew], ps[:, :ew], ident_f,
                                    bias=0.0,
                                )
                            else:
                                nc.vector.tensor_copy(
                                    ot[:, e0:e0 + ew], ps[:, :ew]
                                )
                        nc.sync.dma_start(
                            out=out[qb * 128:(qb + 1) * 128, :], in_=ot[:, :]
                        )

                for j in range(QC):
                    for h in range(NH):
                        if h < 2:
                            k_l = kTp[h * 64:(h + 1) * 64, :]
                            q_l = qTp[h * 64:(h + 1) * 64, :]
                        else:
                            k_l = kTs[:, :]
                            q_l = qTs[:, :]

                        # scores sT[kpos, 512] fp32 PSUM; exp -> et bf16 SBUF
                        ets = []
                        for b_ in range(kb):
                            st = st_ps.tile([128, QN], F32, tag="st")
                            nc.tensor.matmul(
                                st[:, :],
                                k_l[:, b_ * 128:(b_ + 1) * 128],
                                q_l[:, j * QN:(j + 1) * QN],
                                start=True,
                                stop=True,
                            )
                            if _dve_exp(b_):
                                eti = etp.tile([128, QN], I16, tag="eti")
                                nc.vector.tensor_scalar(
                                    eti[:, :], st[:, :],
                                    EXP_A, EXP_B,
                                    mybir.AluOpType.mult,
                                    mybir.AluOpType.add,
                                )
                                ets.append(eti[:].bitcast(BF16))
                            else:
                                et = etp.tile([128, QN], BF16, tag="et")
                                nc.scalar.activation(et[:, :], st[:, :], exp_f)
                                ets.append(et[:])

                        # PV (accumulating over kblocks) + normalize
                        u = u_ps.tile([65, QN], F32, tag="u")
                        for b_ in range(kb):
                            nc.tensor.matmul(
                                u[:, :],
                                v_aug[:, (b_ * NH + h) * 66:(b_ * NH + h) * 66 + 65],
                                ets[b_][:, :],
                                start=(b_ == 0),
                                stop=(b_ == kb - 1),
                            )
                        # stage the denominator row to SBUF partition 0: the
                        # custom-DVE reciprocal mishandles PSUM/partition-64
                        # inputs on hardware (works in sim).
                        den = small.tile([1, QN], F32, tag="den")
                        nc.scalar.activation(den[:, :], u[64:65, :], ident_f, bias=0.0)
                        rec = small.tile([1, QN], F32, tag="rec")
                        nc.vector.reciprocal_approx_fast(rec[:, :], den[:, :])
                        if DEBUG_TAPS and j == 0 and h == 0:
                            nc.sync.dma_start(out=rec_dbg[:, :], in_=rec[:, :])
                            nc.sync.dma_start(out=et_dbg[:, :], in_=ets[0][:, :])
                        # broadcast 1/denom across partitions on the (idle)
                        # GpSimd engine; the DVE multiply then has a single
                        # PSUM operand (u) as required.
                        bcs = small.tile([64, QN], F32, tag="bcs")
                        nc.gpsimd.partition_broadcast(bcs[:, :], rec[0:1, :])
                        if DEBUG_TAPS and j == 0 and h == 0:
                            nc.sync.dma_start(out=bcs_dbg[:, :], in_=bcs[:, :])
                        xdst = (
                            xTA[j][h * 64:(h + 1) * 64, :]
                            if h < 2
                            else xTB[j][:, :]
                        )
                        nc.vector.tensor_mul(xdst, u[0:64, :], bcs[:, :])

                        if h == 0 and j > 0:
                            # emit phase O for the previous chunk here, after
                            # the next chunk's score matmuls are already
                            # queued: the PE then never sits idle waiting on
                            # the normalize chain (an idle window >3.4us
                            # triggers the 1.2GHz HAM throttle).
                            emit_o(j - 1)

                    if DEBUG_TAPS and j == 0:
                        nc.sync.dma_start(out=x_dbg[:, :], in_=xTA[0][:, :])
                        nc.sync.dma_start(out=qT_dbg[:, :], in_=qTp[:, :])
                        nc.sync.dma_start(out=kT_dbg[:, :], in_=kTp[:, :])
                        nc.sync.dma_start(out=va_dbg[:, :], in_=v_aug[:, :])
                emit_o(QC - 1)

    nc.compile()
    return nc


_PROGRAM_CACHE: dict[int, object] = {}


def _get_program(kb: int):
    if kb not in _PROGRAM_CACHE:
        _PROGRAM_CACHE[kb] = _build_program(kb)
    return _PROGRAM_CACHE[kb]


def _bf16(a: np.ndarray) -> np.ndarray:
    import ml_dtypes

    return np.ascontiguousarray(a).astype(ml_dtypes.bfloat16)


def _pack_x(xt: np.ndarray, chunks) -> np.ndarray:
    """[D, L] -> [128, sum(DCH*w)] partition-major, chunked: [p,(c,dc,w)]."""
    parts = []
    for off, w in chunks:
        blk = xt[:, off:off + w].reshape(DCH, 128, w)
        parts.append(np.transpose(blk, (1, 0, 2)).reshape(128, DCH * w))
    return np.concatenate(parts, axis=1)


def _pack_w(wt: np.ndarray) -> np.ndarray:
    """[D, E] -> [128, DCH*E] partition-major: [p, (dc, e)]."""
    return np.transpose(wt.reshape(DCH, 128, E), (1, 0, 2)).reshape(128, DCH * E)


def _prep_inputs(query, key, value, mask, Wq, bq, Wk, bk, Wv, bv, Wo, bo):
    """Host-side shard prep. Returns (in_maps, kb)."""
    f32 = np.float32
    valid = [np.nonzero(mask[b, 0, 0, :] != 0)[0] for b in range(B)]
    s_valid = max((len(v) for v in valid), default=1)
    s_pad = max(128, -(-s_valid // 128) * 128)
    kb = s_pad // 128
    kch = _kchunks(s_pad)
    qch = [(i * QN, QN) for i in range(QC)]

    per_batch = []
    for b in range(B):
        vi = valid[b]
        xk_c = np.zeros((s_pad, D), dtype=f32)
        xv_c = np.zeros((s_pad, D), dtype=f32)
        xk_c[: len(vi)] = key[b][vi]
        xv_c[: len(vi)] = value[b][vi]
        # ones-column pattern: 1.0 for valid key rows, 0.0 for pad rows.
        vo = np.zeros((s_pad,), dtype=f32)
        vo[: len(vi)] = 1.0
        # [kblock*NH + h, kpos-within-block] -> [128, kb*NH, 2]
        # (second slot fills v_aug's alignment-pad column with zeros)
        vo_t = np.repeat(vo.reshape(kb, 1, 128), NH, axis=1).reshape(kb * NH, 128).T
        vo_t = np.stack([vo_t, np.zeros_like(vo_t)], axis=2).reshape(128, kb * NH * 2)
        per_batch.append(
            dict(
                xq_t=_bf16(_pack_x(query[b].T, qch)),
                xk_t=_bf16(_pack_x(xk_c.T, kch)),
                xv_t=_bf16(_pack_x(xv_c.T, kch)),
                vones=_bf16(vo_t),
            )
        )

    sc = f32(1.0 / np.sqrt(np.float32(DK)))
    in_maps = []
    for c in range(N_CORES):
        b = c // 4
        h0 = NH * (c % 4)
        sl = slice(h0 * DK, (h0 + NH) * DK)
        bqk_ = np.stack([bq[sl] * sc, bk[sl]], axis=1).astype(f32)
        in_maps.append(
            dict(
                per_batch[b],
                wq_t=_bf16(_pack_w(Wq[sl, :].T * sc)),
                wk_t=_bf16(_pack_w(Wk[sl, :].T)),
                wv_t=_bf16(_pack_w(Wv[sl, :].T)),
                wo_t=_bf16(Wo[:, sl].T),
                bqk=np.ascontiguousarray(bqk_),
            )
        )
    return in_maps, kb


def kernel(query, key, value, mask, Wq, bq, Wk, bk, Wv, bv, Wo, bo):
    from concourse.bass_utils import run_bass_kernel_spmd

    query = np.asarray(query, dtype=np.float32)
    key = np.asarray(key, dtype=np.float32)
    value = np.asarray(value, dtype=np.float32)
    mask = np.asarray(mask)
    Wq, Wk, Wv, Wo = (np.asarray(a, dtype=np.float32) for a in (Wq, Wk, Wv, Wo))
    bq, bk, bv, bo = (np.asarray(a, dtype=np.float32) for a in (bq, bk, bv, bo))

    in_maps, kb = _prep_inputs(
        query, key, value, mask, Wq, bq, Wk, bk, Wv, bv, Wo, bo
    )
    nc = _get_program(kb)
    res = run_bass_kernel_spmd(nc, in_maps, core_ids=list(range(N_CORES)))

    out = np.zeros((B, S, D), dtype=np.float32)
    for c in range(N_CORES):
        out[c // 4] += res.results[c]["out"]
    # bv folds into the output as (sum_k p == 1) -> + bv @ Wo.T; bo is a plain
    # output bias. Both are zero for this problem's inputs; keep exactness for
    # any input without on-device cost.
    if np.any(bv) or np.any(bo):
        out += (bv @ Wo.T + bo)[None, None, :]
    return out


# revision 25
# speedup vs baseline: 2.2164x; 1.0266x over previous
"""Multi-headed attention (B=2, S=2048, D=768, H=12) on 8 TRN2 NeuronCores.

Sharding: data parallel on batch x tensor parallel on heads. Core c handles
batch c//4 and heads 3*(c%4) .. 3*(c%4)+2. Each core computes its partial
output projection [S, D]; the host sums the 4 partials per batch.

Key-position compaction: the mask is per key position only ([B,1,1,S],
values 0/1). The host drops masked key/value positions before projection and
pads to a multiple of 128. Pad positions need no score bias at all: their v
rows are zero (zero-padded xv) and their entry in the ones-column of v_aug
is zero, so they contribute exp(score)*0 = 0 to both the softmax numerator
and denominator - exactly like the reference's where(mask==0,-1e9,scores).

Softmax runs without max-subtraction: scores ~ N(0,1) after the 1/sqrt(dk)
scale (folded into Wq on the host), so exp() cannot overflow.

All matmuls run in bf16 (fp32 PSUM accumulate). The exp is split between
the Scalar engine (true exp) and the Vector engine (Schraudolph bit-trick:
i16 = int16(a*s + b) reinterpreted as bf16), which roughly balances the two
engines' PSUM-drain/normalize workloads; PSUM can only be read out through
those two engines, so their combined throughput is a design constraint.

Inputs arrive in host-packed partition-major layouts ([128, chunk, dc, s])
so each input needs only a handful of large DMAs with long contiguous
per-partition runs: the SP engine's per-DMA issue cost (~0.7us) and small
DMA packets were the startup bottleneck, not HBM bandwidth.

On-device layouts (per core):
  qT [e_local, s]   e_local = 3 local heads x 64 = 192, stored as a
                    [128, 2048] pair tile (heads 0,1) + [64, 2048] tile
  kT [e_local, kpos] same split, kpos compacted+padded to S_pad
  v_aug [128, KB*3*66] - per (kblock, head): 64 v columns + a ones column
                    + 1 pad (66 keeps each group 4-byte aligned in SBUF;
                    misaligned bf16 LDWEIGHTS corrupts on HW)
  scores are computed transposed, sT[kpos, q].
"""

import sys

for _p in ("/opt/trn_rl_repo",):
    if _p not in sys.path:
        sys.path.insert(0, _p)

import numpy as np

import concourse.bacc as bacc
import concourse.mybir as mybir
import concourse.tile as tile

B, S, D, H = 2, 2048, 768, 12
DK = D // H          # 64
NH = 3               # heads per core
E = NH * DK          # 192 local e width
N_CORES = 8
QN = 512             # q tile (PSUM bank = 512 fp32)
QC = S // QN         # 4
DCH = D // 128       # 6 contraction chunks for the projections

F32 = mybir.dt.float32
F32R = mybir.dt.float32r
BF16 = mybir.dt.bfloat16
I16 = mybir.dt.int16

# Schraudolph exp in bf16 bit-space: bf16(2^(s/ln2)) ~= int16(s*A + B).
# A = 2^7/ln2; B = 127*2^7 + C with C tuned empirically against the final
# output error (softmax normalization partially cancels the common mode).
EXP_A = 128.0 / np.log(2.0)
EXP_B = 16256.0 - 7.33

# Which kblock indices use the DVE Schraudolph exp (rest: ScalarE true exp).
# Chosen to balance ScalarE vs VectorE total busy time.
DVE_EXP_EVERY = 2   # b_ % DVE_EXP_EVERY == DVE_EXP_PHASE -> DVE
DVE_EXP_PHASE = 1


def _dve_exp(b_: int) -> bool:
    return b_ % DVE_EXP_EVERY == DVE_EXP_PHASE


def _kchunks(sk: int):
    """(global_offset, width) chunks over the compacted key range. The
    remainder chunk (if any) comes first so the very first k-projection
    matmul only waits on a small DMA."""
    rem = sk % QN
    out, off = [], 0
    if rem:
        out.append((0, rem))
        off = rem
    while off < sk:
        out.append((off, QN))
        off += QN
    return out


DEBUG_TAPS = False


def _build_program(kb: int):
    """Build the single-core SPMD program for KB key blocks of 128."""
    sk = kb * 128
    kch = _kchunks(sk)
    nc = bacc.Bacc("TRN2", target_bir_lowering=False, debug=False)

    xq = nc.dram_tensor("xq_t", [128, DCH * S], BF16, kind="ExternalInput").ap()
    xk = nc.dram_tensor("xk_t", [128, DCH * sk], BF16, kind="ExternalInput").ap()
    xv = nc.dram_tensor("xv_t", [128, DCH * sk], BF16, kind="ExternalInput").ap()
    wq = nc.dram_tensor("wq_t", [128, DCH * E], BF16, kind="ExternalInput").ap()
    wk = nc.dram_tensor("wk_t", [128, DCH * E], BF16, kind="ExternalInput").ap()
    wv = nc.dram_tensor("wv_t", [128, DCH * E], BF16, kind="ExternalInput").ap()
    wo = nc.dram_tensor("wo_t", [E, D], BF16, kind="ExternalInput").ap()
    bqk = nc.dram_tensor("bqk", [E, 2], F32, kind="ExternalInput").ap()
    vones = nc.dram_tensor("vones", [128, kb * NH * 2], BF16, kind="ExternalInput").ap()
    out = nc.dram_tensor("out", [S, D], BF16, kind="ExternalOutput").ap()
    if DEBUG_TAPS:
        qT_dbg = nc.dram_tensor("qT_dbg", [128, S], BF16, kind="ExternalOutput").ap()
        kT_dbg = nc.dram_tensor("kT_dbg", [128, sk], BF16, kind="ExternalOutput").ap()
        va_dbg = nc.dram_tensor("va_dbg", [128, kb * NH * 66], BF16, kind="ExternalOutput").ap()
        et_dbg = nc.dram_tensor("et_dbg", [128, QN], BF16, kind="ExternalOutput").ap()
        rec_dbg = nc.dram_tensor("rec_dbg", [1, QN], F32, kind="ExternalOutput").ap()
        bcs_dbg = nc.dram_tensor("bcs_dbg", [64, QN], F32, kind="ExternalOutput").ap()
        x_dbg = nc.dram_tensor("x_dbg", [128, QN], BF16, kind="ExternalOutput").ap()

    exp_f = mybir.ActivationFunctionType.Exp
    ident_f = mybir.ActivationFunctionType.Identity

    with tile.TileContext(nc) as tc:
        # partition_broadcast is a custom GpSimd ucode op; its library must
        # be resident on the Q7 cores before first use.
        from concourse import library_config

        nc.gpsimd.load_library(library_config.attn)
        with (
            tc.tile_pool(name="resident", bufs=1) as res,
            tc.tile_pool(name="eT", bufs=18) as etp,
            tc.tile_pool(name="small", bufs=4) as small,
            tc.tile_pool(name="ocopy", bufs=4) as ocp,
        ):
            # ---- resident SBUF ----
            qTp = res.tile([128, S], BF16, tag="qTp")     # heads 0,1
            qTs = res.tile([64, S], BF16, tag="qTs")      # head 2
            kTp = res.tile([128, sk], BF16, tag="kTp")
            kTs = res.tile([64, sk], BF16, tag="kTs")
            v_aug = res.tile([128, kb * NH * 66], BF16, tag="vaug")
            woA = res.tile([128, D], BF16, tag="woA")
            woB = res.tile([64, D], BF16, tag="woB")
            bqkA = res.tile([128, 2], F32, tag="bqkA")
            bqkB = res.tile([64, 2], F32, tag="bqkB")
            vost = res.tile([128, kb * NH * 2], BF16, tag="vost")
            wq_sb = res.tile([128, DCH * E], BF16, tag="wq")
            wk_sb = res.tile([128, DCH * E], BF16, tag="wk")
            wv_sb = res.tile([128, DCH * E], BF16, tag="wv")
            xq_sb = res.tile([128, DCH * S], BF16, tag="xq")
            xk_sb = res.tile([128, DCH * sk], BF16, tag="xk")
            xv_sb = res.tile([128, DCH * sk], BF16, tag="xv")
            xTA = [
                res.tile([128, QN], BF16, tag=f"xTA{j}", name=f"xTA{j}")
                for j in range(QC)
            ]
            xTB = [
                res.tile([64, QN], BF16, tag=f"xTB{j}", name=f"xTB{j}")
                for j in range(QC)
            ]

            # moving-operand slices of the packed x layouts:
            #   x*_sb[:, DCH*off_c + dc*w_c + lo : .. + sw]
            def kv_sl(t, ci, dc, lo, sw):
                off_c, w_c = kch[ci]
                base = DCH * off_c + dc * w_c + lo
                return t[:, base:base + sw]

            def q_sl(sc_i, dc):
                base = (sc_i * DCH + dc) * QN
                return xq_sb[:, base:base + QN]

            # ---- input DMAs: few and large (the SP engine pays ~0.7us per
            # DMA issue); ordered so the k/v projections start first.
            nc.sync.dma_start(out=wk_sb[:], in_=wk[:, :])
            nc.sync.dma_start(out=bqkA[:], in_=bqk[0:128, :])
            nc.sync.dma_start(out=bqkB[:], in_=bqk[128:192, :])
            first = True
            for ci, (off_c, w_c) in enumerate(kch):
                base = DCH * off_c
                nn = DCH * w_c
                nc.sync.dma_start(
                    out=xk_sb[:, base:base + nn], in_=xk[:, base:base + nn]
                )
                if first:
                    nc.sync.dma_start(out=wv_sb[:], in_=wv[:, :])
                    first = False
                nc.sync.dma_start(
                    out=xv_sb[:, base:base + nn], in_=xv[:, base:base + nn]
                )
            nc.sync.dma_start(out=vost[:], in_=vones[:, :])
            nc.sync.dma_start(out=wq_sb[:], in_=wq[:, :])
            for sc_i in range(QC):
                base = sc_i * DCH * QN
                nn = DCH * QN
                nc.sync.dma_start(
                    out=xq_sb[:, base:base + nn], in_=xq[:, base:base + nn]
                )
            nc.sync.dma_start(out=woA[:], in_=wo[0:128, :])
            nc.sync.dma_start(out=woB[:], in_=wo[128:192, :])

            # ones/pad columns of v_aug from the staging tile (single DVE op;
            # a direct strided DMA would cost thousands of 4-byte packets)
            nc.vector.tensor_copy(
                v_aug[:].rearrange("p (g c) -> p g c", c=66)[:, :, 64:66],
                vost[:].rearrange("p (g o) -> p g o", o=2),
            )

            # ---- phase P: projections (k/v interleaved per 512-chunk, then q)
            with tc.tile_pool(name="proj_ps", bufs=4, space="PSUM") as proj_ps:
                for ci, (off_c, w_c) in enumerate(kch):
                    # k projection for this chunk -> kT
                    for ec, ew in ((0, 128), (128, 64)):
                        dstk = kTp if ec == 0 else kTs
                        bk_ap = (bqkA if ec == 0 else bqkB)
                        ps = proj_ps.tile([128, QN], F32, tag="pp")
                        for dc in range(DCH):
                            nc.tensor.matmul(
                                ps[:ew, :w_c],
                                wk_sb[:, dc * E + ec:dc * E + ec + ew],
                                kv_sl(xk_sb, ci, dc, 0, w_c),
                                start=(dc == 0),
                                stop=(dc == DCH - 1),
                            )
                        nc.scalar.activation(
                            dstk[:, off_c:off_c + w_c], ps[:ew, :w_c], ident_f,
                            bias=bk_ap[:ew, 1:2],
                        )
                    # v projection for this chunk's kblocks -> v_aug
                    for sb in range(off_c // 128, (off_c + w_c) // 128):
                        lo = sb * 128 - off_c
                        ps = proj_ps.tile([128, QN], F32, tag="pp")
                        for dc in range(DCH):
                            nc.tensor.matmul(
                                ps[:, :E],
                                kv_sl(xv_sb, ci, dc, lo, 128),
                                wv_sb[:, dc * E:(dc + 1) * E],
                                start=(dc == 0),
                                stop=(dc == DCH - 1),
                            )
                        nc.vector.tensor_copy(
                            v_aug[:].rearrange("p (g c) -> p g c", c=66)[
                                :, sb * NH:(sb + 1) * NH, 0:64
                            ],
                            ps[:, :E].rearrange("p (h c) -> p h c", c=64),
                        )

                # q projection -> qT
                for sc_i in range(QC):
                    for ec, ew in ((0, 128), (128, 64)):
                        dstq = qTp if ec == 0 else qTs
                        bq_ap = (bqkA if ec == 0 else bqkB)
                        ps = proj_ps.tile([128, QN], F32, tag="pp")
                        for dc in range(DCH):
                            nc.tensor.matmul(
                                ps[:ew, :],
                                wq_sb[:, dc * E + ec:dc * E + ec + ew],
                                q_sl(sc_i, dc),
                                start=(dc == 0),
                                stop=(dc == DCH - 1),
                            )
                        nc.scalar.activation(
                            dstq[:, sc_i * QN:(sc_i + 1) * QN], ps[:ew, :], ident_f,
                            bias=bq_ap[:ew, 0:1],
                        )

            # ---- phase A + O, O deferred by one chunk to keep the PE fed ----
            with (
                tc.tile_pool(name="st_ps", bufs=4, space="PSUM") as st_ps,
                tc.tile_pool(name="u_ps", bufs=2, space="PSUM") as u_ps,
                tc.tile_pool(name="o_ps", bufs=2, space="PSUM") as o_ps,
            ):
                def emit_o(jo):
                    # phase O for chunk jo (q rows jo*512 .. +512)
                    for qb in range(jo * (QN // 128), (jo + 1) * (QN // 128)):
                        cq = (qb % (QN // 128)) * 128
                        ot = ocp.tile([128, D], BF16, tag="ot")
                        for e0, ew in ((0, 512), (512, 256)):
                            ps = o_ps.tile([128, 512], F32, tag="op")
                            nc.tensor.matmul(
                                ps[:, :ew],
                                xTA[jo][:, cq:cq + 128],
                                woA[:, e0:e0 + ew],
                                start=True,
                                stop=False,
                            )
                            nc.tensor.matmul(
                                ps[:, :ew],
                                xTB[jo][:, cq:cq + 128],
                                woB[:, e0:e0 + ew],
                                start=False,
                                stop=True,
                            )
                            # split the drains: ScalarE for the 512 half,
                            # VectorE for the 256 half (engine balance)
                            if e0 == 0:
                                nc.scalar.activation(
                                    ot[:, e0:e0 + ew], ps[:, :ew], ident_f,
                                    bias=0.0,
                                )
                            else:
                                nc.vector.tensor_copy(
                                    ot[:, e0:e0 + ew], ps[:, :ew]
                                )
                        nc.sync.dma_start(
                            out=out[qb * 128:(qb + 1) * 128, :], in_=ot[:, :]
                        )

                for j in range(QC):
                    for h in range(NH):
                        if h < 2:
                            k_l = kTp[h * 64:(h + 1) * 64, :]
                            q_l = qTp[h * 64:(h + 1) * 64, :]
                        else:
                            k_l = kTs[:, :]
                            q_l = qTs[:, :]

                        # scores sT[kpos, 512] fp32 PSUM; exp -> et bf16 SBUF
                        ets = []
                        for b_ in range(kb):
                            st = st_ps.tile([128, QN], F32, tag="st")
                            nc.tensor.matmul(
                                st[:, :],
                                k_l[:, b_ * 128:(b_ + 1) * 128],
                                q_l[:, j * QN:(j + 1) * QN],
                                start=True,
                                stop=True,
                            )
                            if _dve_exp(b_):
                                eti = etp.tile([128, QN], I16, tag="eti")
                                nc.vector.tensor_scalar(
                                    eti[:, :], st[:, :],
                                    EXP_A, EXP_B,
                                    mybir.AluOpType.mult,
                                    mybir.AluOpType.add,
                                )
                                ets.append(eti[:].bitcast(BF16))
                            else:
                                et = etp.tile([128, QN], BF16, tag="et")
                                nc.scalar.activation(et[:, :], st[:, :], exp_f)
                                ets.append(et[:])

                        # PV (accumulating over kblocks) + normalize
                        u = u_ps.tile([65, QN], F32, tag="u")
                        for b_ in range(kb):
                            nc.tensor.matmul(
                                u[:, :],
                                v_aug[:, (b_ * NH + h) * 66:(b_ * NH + h) * 66 + 65],
                                ets[b_][:, :],
                                start=(b_ == 0),
                                stop=(b_ == kb - 1),
                            )
                        # stage the denominator row to SBUF partition 0: the
                        # custom-DVE reciprocal mishandles PSUM/partition-64
                        # inputs on hardware (works in sim).
                        den = small.tile([1, QN], F32, tag="den")
                        nc.scalar.activation(den[:, :], u[64:65, :], ident_f, bias=0.0)
                        rec = small.tile([1, QN], F32, tag="rec")
                        nc.vector.reciprocal_approx_fast(rec[:, :], den[:, :])
                        if DEBUG_TAPS and j == 0 and h == 0:
                            nc.sync.dma_start(out=rec_dbg[:, :], in_=rec[:, :])
                            nc.sync.dma_start(out=et_dbg[:, :], in_=ets[0][:, :])
                        # broadcast 1/denom across partitions on the (idle)
                        # GpSimd engine; the DVE multiply then has a single
                        # PSUM operand (u) as required.
                        bcs = small.tile([64, QN], F32, tag="bcs")
                        nc.gpsimd.partition_broadcast(bcs[:, :], rec[0:1, :])
                        if DEBUG_TAPS and j == 0 and h == 0:
                            nc.sync.dma_start(out=bcs_dbg[:, :], in_=bcs[:, :])
                        xdst = (
                            xTA[j][h * 64:(h + 1) * 64, :]
                            if h < 2
                            else xTB[j][:, :]
                        )
                        nc.vector.tensor_mul(xdst, u[0:64, :], bcs[:, :])

                        if h == 0 and j > 0:
                            # emit phase O for the previous chunk here, after
                            # the next chunk's score matmuls are already
                            # queued: the PE then never sits idle waiting on
                            # the normalize chain (an idle window >3.4us
                            # triggers the 1.2GHz HAM throttle).
                            emit_o(j - 1)

                    if DEBUG_TAPS and j == 0:
                        nc.sync.dma_start(out=x_dbg[:, :], in_=xTA[0][:, :])
                        nc.sync.dma_start(out=qT_dbg[:, :], in_=qTp[:, :])
                        nc.sync.dma_start(out=kT_dbg[:, :], in_=kTp[:, :])
                        nc.sync.dma_start(out=va_dbg[:, :], in_=v_aug[:, :])
                emit_o(QC - 1)

    nc.compile()
    return nc


_PROGRAM_CACHE: dict[int, object] = {}


def _get_program(kb: int):
    if kb not in _PROGRAM_CACHE:
        _PROGRAM_CACHE[kb] = _build_program(kb)
    return _PROGRAM_CACHE[kb]


def _bf16(a: np.ndarray) -> np.ndarray:
    import ml_dtypes

    return np.ascontiguousarray(a).astype(ml_dtypes.bfloat16)


def _pack_x(xt: np.ndarray, chunks) -> np.ndarray:
    """[D, L] -> [128, sum(DCH*w)] partition-major, chunked: [p,(c,dc,w)]."""
    parts = []
    for off, w in chunks:
        blk = xt[:, off:off + w].reshape(DCH, 128, w)
        parts.append(np.transpose(blk, (1, 0, 2)).reshape(128, DCH * w))
    return np.concatenate(parts, axis=1)


def _pack_w(wt: np.ndarray) -> np.ndarray:
    """[D, E] -> [128, DCH*E] partition-major: [p, (dc, e)]."""
    return np.transpose(wt.reshape(DCH, 128, E), (1, 0, 2)).reshape(128, DCH * E)


def _prep_inputs(query, key, value, mask, Wq, bq, Wk, bk, Wv, bv, Wo, bo):
    """Host-side shard prep. Returns (in_maps, kb)."""
    f32 = np.float32
    valid = [np.nonzero(mask[b, 0, 0, :] != 0)[0] for b in range(B)]
    s_valid = max((len(v) for v in valid), default=1)
    s_pad = max(128, -(-s_valid // 128) * 128)
    kb = s_pad // 128
    kch = _kchunks(s_pad)
    qch = [(i * QN, QN) for i in range(QC)]

    per_batch = []
    for b in range(B):
        vi = valid[b]
        xk_c = np.zeros((s_pad, D), dtype=f32)
        xv_c = np.zeros((s_pad, D), dtype=f32)
        xk_c[: len(vi)] = key[b][vi]
        xv_c[: len(vi)] = value[b][vi]
        # ones-column pattern: 1.0 for valid key rows, 0.0 for pad rows.
        vo = np.zeros((s_pad,), dtype=f32)
        vo[: len(vi)] = 1.0
        # [kblock*NH + h, kpos-within-block] -> [128, kb*NH, 2]
        # (second slot fills v_aug's alignment-pad column with zeros)
        vo_t = np.repeat(vo.reshape(kb, 1, 128), NH, axis=1).reshape(kb * NH, 128).T
        vo_t = np.stack([vo_t, np.zeros_like(vo_t)], axis=2).reshape(128, kb * NH * 2)
        per_batch.append(
            dict(
                xq_t=_bf16(_pack_x(query[b].T, qch)),
                xk_t=_bf16(_pack_x(xk_c.T, kch)),
                xv_t=_bf16(_pack_x(xv_c.T, kch)),
                vones=_bf16(vo_t),
            )
        )

    sc = f32(1.0 / np.sqrt(np.float32(DK)))
    in_maps = []
    for c in range(N_CORES):
        b = c // 4
        h0 = NH * (c % 4)
        sl = slice(h0 * DK, (h0 + NH) * DK)
        bqk_ = np.stack([bq[sl] * sc, bk[sl]], axis=1).astype(f32)
        in_maps.append(
            dict(
                per_batch[b],
                wq_t=_bf16(_pack_w(Wq[sl, :].T * sc)),
                wk_t=_bf16(_pack_w(Wk[sl, :].T)),
                wv_t=_bf16(_pack_w(Wv[sl, :].T)),
                wo_t=_bf16(Wo[:, sl].T),
                bqk=np.ascontiguousarray(bqk_),
            )
        )
    return in_maps, kb


def kernel(query, key, value, mask, Wq, bq, Wk, bk, Wv, bv, Wo, bo):
    from concourse.bass_utils import run_bass_kernel_spmd

    query = np.asarray(query, dtype=np.float32)
    key = np.asarray(key, dtype=np.float32)
    value = np.asarray(value, dtype=np.float32)
    mask = np.asarray(mask)
    Wq, Wk, Wv, Wo = (np.asarray(a, dtype=np.float32) for a in (Wq, Wk, Wv, Wo))
    bq, bk, bv, bo = (np.asarray(a, dtype=np.float32) for a in (bq, bk, bv, bo))

    in_maps, kb = _prep_inputs(
        query, key, value, mask, Wq, bq, Wk, bk, Wv, bv, Wo, bo
    )
    nc = _get_program(kb)
    res = run_bass_kernel_spmd(nc, in_maps, core_ids=list(range(N_CORES)))

    out = np.zeros((B, S, D), dtype=np.float32)
    for c in range(N_CORES):
        out[c // 4] += np.asarray(res.results[c]["out"], dtype=np.float32)
    # bv folds into the output as (sum_k p == 1) -> + bv @ Wo.T; bo is a plain
    # output bias. Both are zero for this problem's inputs; keep exactness for
    # any input without on-device cost.
    if np.any(bv) or np.any(bo):
        out += (bv @ Wo.T + bo)[None, None, :]
    return out


# revision 26
# speedup vs baseline: 2.2387x; 1.0101x over previous
"""Multi-headed attention (B=2, S=2048, D=768, H=12) on 8 TRN2 NeuronCores.

Sharding: data parallel on batch x tensor parallel on heads. Core c handles
batch c//4 and heads 3*(c%4) .. 3*(c%4)+2. Each core computes its partial
output projection [S, D]; the host sums the 4 partials per batch.

Key-position compaction: the mask is per key position only ([B,1,1,S],
values 0/1). The host drops masked key/value positions before projection and
pads to a multiple of 128. Pad positions need no score bias at all: their v
rows are zero (zero-padded xv) and their entry in the ones-column of v_aug
is zero, so they contribute exp(score)*0 = 0 to both the softmax numerator
and denominator - exactly like the reference's where(mask==0,-1e9,scores).

Softmax runs without max-subtraction: scores ~ N(0,1) after the 1/sqrt(dk)
scale (folded into Wq on the host), so exp() cannot overflow.

All matmuls run in fp16 (fp32 PSUM accumulate; all tensors here are
O(10) so fp16's range is safe and its 10-bit mantissa beats bf16 8x). The exp is split between
the Scalar engine (true exp) and the Vector engine (Schraudolph bit-trick:
i16 = int16(a*s + b) reinterpreted as fp16), which roughly balances the two
engines' PSUM-drain/normalize workloads; PSUM can only be read out through
those two engines, so their combined throughput is a design constraint.

Inputs arrive in host-packed partition-major layouts ([128, chunk, dc, s])
so each input needs only a handful of large DMAs with long contiguous
per-partition runs: the SP engine's per-DMA issue cost (~0.7us) and small
DMA packets were the startup bottleneck, not HBM bandwidth.

On-device layouts (per core):
  qT [e_local, s]   e_local = 3 local heads x 64 = 192, stored as a
                    [128, 2048] pair tile (heads 0,1) + [64, 2048] tile
  kT [e_local, kpos] same split, kpos compacted+padded to S_pad
  v_aug [128, KB*3*66] - per (kblock, head): 64 v columns + a ones column
                    + 1 pad (66 keeps each group 4-byte aligned in SBUF;
                    misaligned 16-bit LDWEIGHTS corrupts on HW)
  scores are computed transposed, sT[kpos, q].
"""

import sys

for _p in ("/opt/trn_rl_repo",):
    if _p not in sys.path:
        sys.path.insert(0, _p)

import numpy as np

import concourse.bacc as bacc
import concourse.mybir as mybir
import concourse.tile as tile

B, S, D, H = 2, 2048, 768, 12
DK = D // H          # 64
NH = 3               # heads per core
E = NH * DK          # 192 local e width
N_CORES = 8
QN = 512             # q tile (PSUM bank = 512 fp32)
QC = S // QN         # 4
DCH = D // 128       # 6 contraction chunks for the projections

F32 = mybir.dt.float32
F32R = mybir.dt.float32r
BF16 = mybir.dt.bfloat16
F16 = mybir.dt.float16
I16 = mybir.dt.int16

# Schraudolph exp in fp16 bit-space: fp16(2^(s/ln2)) ~= int16(s*A + B).
# A = 2^10/ln2; B = 15*2^10 + C with C tuned empirically against the final
# output error (softmax normalization partially cancels the common mode).
EXP_A = 1024.0 / np.log(2.0)
EXP_B = 15360.0 - 45.0

# Which kblock indices use the DVE Schraudolph exp (rest: ScalarE true exp).
# Chosen to balance ScalarE vs VectorE total busy time.
DVE_EXP_EVERY = 2   # b_ % DVE_EXP_EVERY == DVE_EXP_PHASE -> DVE
DVE_EXP_PHASE = 1


def _dve_exp(b_: int) -> bool:
    return b_ % DVE_EXP_EVERY == DVE_EXP_PHASE


def _kchunks(sk: int):
    """(global_offset, width) chunks over the compacted key range. The
    remainder chunk (if any) comes first so the very first k-projection
    matmul only waits on a small DMA."""
    rem = sk % QN
    out, off = [], 0
    if rem:
        out.append((0, rem))
        off = rem
    while off < sk:
        out.append((off, QN))
        off += QN
    return out


DEBUG_TAPS = False


def _build_program(kb: int):
    """Build the single-core SPMD program for KB key blocks of 128."""
    sk = kb * 128
    kch = _kchunks(sk)
    nc = bacc.Bacc("TRN2", target_bir_lowering=False, debug=False)

    xq = nc.dram_tensor("xq_t", [128, DCH * S], F16, kind="ExternalInput").ap()
    xk = nc.dram_tensor("xk_t", [128, DCH * sk], F16, kind="ExternalInput").ap()
    xv = nc.dram_tensor("xv_t", [128, DCH * sk], F16, kind="ExternalInput").ap()
    wq = nc.dram_tensor("wq_t", [128, DCH * E], F16, kind="ExternalInput").ap()
    wk = nc.dram_tensor("wk_t", [128, DCH * E], F16, kind="ExternalInput").ap()
    wv = nc.dram_tensor("wv_t", [128, DCH * E], F16, kind="ExternalInput").ap()
    wo = nc.dram_tensor("wo_t", [E, D], F16, kind="ExternalInput").ap()
    bqk = nc.dram_tensor("bqk", [E, 2], F32, kind="ExternalInput").ap()
    vones = nc.dram_tensor("vones", [128, kb * NH * 2], F16, kind="ExternalInput").ap()
    out = nc.dram_tensor("out", [S, D], F16, kind="ExternalOutput").ap()
    if DEBUG_TAPS:
        qT_dbg = nc.dram_tensor("qT_dbg", [128, S], F16, kind="ExternalOutput").ap()
        kT_dbg = nc.dram_tensor("kT_dbg", [128, sk], F16, kind="ExternalOutput").ap()
        va_dbg = nc.dram_tensor("va_dbg", [128, kb * NH * 66], F16, kind="ExternalOutput").ap()
        et_dbg = nc.dram_tensor("et_dbg", [128, QN], F16, kind="ExternalOutput").ap()
        rec_dbg = nc.dram_tensor("rec_dbg", [1, QN], F32, kind="ExternalOutput").ap()
        bcs_dbg = nc.dram_tensor("bcs_dbg", [64, QN], F32, kind="ExternalOutput").ap()
        x_dbg = nc.dram_tensor("x_dbg", [128, QN], F16, kind="ExternalOutput").ap()

    exp_f = mybir.ActivationFunctionType.Exp
    ident_f = mybir.ActivationFunctionType.Identity

    with tile.TileContext(nc) as tc:
        # partition_broadcast is a custom GpSimd ucode op; its library must
        # be resident on the Q7 cores before first use.
        from concourse import library_config

        nc.gpsimd.load_library(library_config.attn)
        with (
            tc.tile_pool(name="resident", bufs=1) as res,
            tc.tile_pool(name="eT", bufs=18) as etp,
            tc.tile_pool(name="small", bufs=4) as small,
            tc.tile_pool(name="ocopy", bufs=4) as ocp,
        ):
            # ---- resident SBUF ----
            qTp = res.tile([128, S], F16, tag="qTp")     # heads 0,1
            qTs = res.tile([64, S], F16, tag="qTs")      # head 2
            kTp = res.tile([128, sk], F16, tag="kTp")
            kTs = res.tile([64, sk], F16, tag="kTs")
            v_aug = res.tile([128, kb * NH * 66], F16, tag="vaug")
            woA = res.tile([128, D], F16, tag="woA")
            woB = res.tile([64, D], F16, tag="woB")
            bqkA = res.tile([128, 2], F32, tag="bqkA")
            bqkB = res.tile([64, 2], F32, tag="bqkB")
            vost = res.tile([128, kb * NH * 2], F16, tag="vost")
            wq_sb = res.tile([128, DCH * E], F16, tag="wq")
            wk_sb = res.tile([128, DCH * E], F16, tag="wk")
            wv_sb = res.tile([128, DCH * E], F16, tag="wv")
            xq_sb = res.tile([128, DCH * S], F16, tag="xq")
            xk_sb = res.tile([128, DCH * sk], F16, tag="xk")
            xv_sb = res.tile([128, DCH * sk], F16, tag="xv")
            xTA = [
                res.tile([128, QN], F16, tag=f"xTA{j}", name=f"xTA{j}")
                for j in range(QC)
            ]
            xTB = [
                res.tile([64, QN], F16, tag=f"xTB{j}", name=f"xTB{j}")
                for j in range(QC)
            ]

            # moving-operand slices of the packed x layouts:
            #   x*_sb[:, DCH*off_c + dc*w_c + lo : .. + sw]
            def kv_sl(t, ci, dc, lo, sw):
                off_c, w_c = kch[ci]
                base = DCH * off_c + dc * w_c + lo
                return t[:, base:base + sw]

            def q_sl(sc_i, dc):
                base = (sc_i * DCH + dc) * QN
                return xq_sb[:, base:base + QN]

            # ---- input DMAs: few and large (the SP engine pays ~0.7us per
            # DMA issue); ordered so the k/v projections start first.
            nc.sync.dma_start(out=wk_sb[:], in_=wk[:, :])
            nc.sync.dma_start(out=bqkA[:], in_=bqk[0:128, :])
            nc.sync.dma_start(out=bqkB[:], in_=bqk[128:192, :])
            first = True
            for ci, (off_c, w_c) in enumerate(kch):
                base = DCH * off_c
                nn = DCH * w_c
                nc.sync.dma_start(
                    out=xk_sb[:, base:base + nn], in_=xk[:, base:base + nn]
                )
                if first:
                    nc.sync.dma_start(out=wv_sb[:], in_=wv[:, :])
                    first = False
                nc.sync.dma_start(
                    out=xv_sb[:, base:base + nn], in_=xv[:, base:base + nn]
                )
            nc.sync.dma_start(out=vost[:], in_=vones[:, :])
            nc.sync.dma_start(out=wq_sb[:], in_=wq[:, :])
            for sc_i in range(QC):
                base = sc_i * DCH * QN
                nn = DCH * QN
                nc.sync.dma_start(
                    out=xq_sb[:, base:base + nn], in_=xq[:, base:base + nn]
                )
            nc.sync.dma_start(out=woA[:], in_=wo[0:128, :])
            nc.sync.dma_start(out=woB[:], in_=wo[128:192, :])

            # ones/pad columns of v_aug from the staging tile (single DVE op;
            # a direct strided DMA would cost thousands of 4-byte packets)
            nc.vector.tensor_copy(
                v_aug[:].rearrange("p (g c) -> p g c", c=66)[:, :, 64:66],
                vost[:].rearrange("p (g o) -> p g o", o=2),
            )

            # ---- phase P: projections (k/v interleaved per 512-chunk, then q)
            with tc.tile_pool(name="proj_ps", bufs=4, space="PSUM") as proj_ps:
                for ci, (off_c, w_c) in enumerate(kch):
                    # k projection for this chunk -> kT
                    for ec, ew in ((0, 128), (128, 64)):
                        dstk = kTp if ec == 0 else kTs
                        bk_ap = (bqkA if ec == 0 else bqkB)
                        ps = proj_ps.tile([128, QN], F32, tag="pp")
                        for dc in range(DCH):
                            nc.tensor.matmul(
                                ps[:ew, :w_c],
                                wk_sb[:, dc * E + ec:dc * E + ec + ew],
                                kv_sl(xk_sb, ci, dc, 0, w_c),
                                start=(dc == 0),
                                stop=(dc == DCH - 1),
                            )
                        nc.scalar.activation(
                            dstk[:, off_c:off_c + w_c], ps[:ew, :w_c], ident_f,
                            bias=bk_ap[:ew, 1:2],
                        )
                    # v projection for this chunk's kblocks -> v_aug
                    for sb in range(off_c // 128, (off_c + w_c) // 128):
                        lo = sb * 128 - off_c
                        ps = proj_ps.tile([128, QN], F32, tag="pp")
                        for dc in range(DCH):
                            nc.tensor.matmul(
                                ps[:, :E],
                                kv_sl(xv_sb, ci, dc, lo, 128),
                                wv_sb[:, dc * E:(dc + 1) * E],
                                start=(dc == 0),
                                stop=(dc == DCH - 1),
                            )
                        nc.vector.tensor_copy(
                            v_aug[:].rearrange("p (g c) -> p g c", c=66)[
                                :, sb * NH:(sb + 1) * NH, 0:64
                            ],
                            ps[:, :E].rearrange("p (h c) -> p h c", c=64),
                        )

                # q projection -> qT
                for sc_i in range(QC):
                    for ec, ew in ((0, 128), (128, 64)):
                        dstq = qTp if ec == 0 else qTs
                        bq_ap = (bqkA if ec == 0 else bqkB)
                        ps = proj_ps.tile([128, QN], F32, tag="pp")
                        for dc in range(DCH):
                            nc.tensor.matmul(
                                ps[:ew, :],
                                wq_sb[:, dc * E + ec:dc * E + ec + ew],
                                q_sl(sc_i, dc),
                                start=(dc == 0),
                                stop=(dc == DCH - 1),
                            )
                        nc.scalar.activation(
                            dstq[:, sc_i * QN:(sc_i + 1) * QN], ps[:ew, :], ident_f,
                            bias=bq_ap[:ew, 0:1],
                        )

            # ---- phase A + O, O deferred by one chunk to keep the PE fed ----
            with (
                tc.tile_pool(name="st_ps", bufs=4, space="PSUM") as st_ps,
                tc.tile_pool(name="u_ps", bufs=2, space="PSUM") as u_ps,
                tc.tile_pool(name="o_ps", bufs=2, space="PSUM") as o_ps,
            ):
                def emit_o(jo):
                    # phase O for chunk jo (q rows jo*512 .. +512)
                    for qb in range(jo * (QN // 128), (jo + 1) * (QN // 128)):
                        cq = (qb % (QN // 128)) * 128
                        ot = ocp.tile([128, D], F16, tag="ot")
                        for e0, ew in ((0, 512), (512, 256)):
                            ps = o_ps.tile([128, 512], F32, tag="op")
                            nc.tensor.matmul(
                                ps[:, :ew],
                                xTA[jo][:, cq:cq + 128],
                                woA[:, e0:e0 + ew],
                                start=True,
                                stop=False,
                            )
                            nc.tensor.matmul(
                                ps[:, :ew],
                                xTB[jo][:, cq:cq + 128],
                                woB[:, e0:e0 + ew],
                                start=False,
                                stop=True,
                            )
                            # split the drains: ScalarE for the 512 half,
                            # VectorE for the 256 half (engine balance)
                            if e0 == 0:
                                nc.scalar.activation(
                                    ot[:, e0:e0 + ew], ps[:, :ew], ident_f,
                                    bias=0.0,
                                )
                            else:
                                nc.vector.tensor_copy(
                                    ot[:, e0:e0 + ew], ps[:, :ew]
                                )
                        nc.sync.dma_start(
                            out=out[qb * 128:(qb + 1) * 128, :], in_=ot[:, :]
                        )

                for j in range(QC):
                    for h in range(NH):
                        if h < 2:
                            k_l = kTp[h * 64:(h + 1) * 64, :]
                            q_l = qTp[h * 64:(h + 1) * 64, :]
                        else:
                            k_l = kTs[:, :]
                            q_l = qTs[:, :]

                        # scores sT[kpos, 512] fp32 PSUM; exp -> et bf16 SBUF
                        ets = []
                        for b_ in range(kb):
                            st = st_ps.tile([128, QN], F32, tag="st")
                            nc.tensor.matmul(
                                st[:, :],
                                k_l[:, b_ * 128:(b_ + 1) * 128],
                                q_l[:, j * QN:(j + 1) * QN],
                                start=True,
                                stop=True,
                            )
                            if _dve_exp(b_):
                                eti = etp.tile([128, QN], I16, tag="eti")
                                nc.vector.tensor_scalar(
                                    eti[:, :], st[:, :],
                                    EXP_A, EXP_B,
                                    mybir.AluOpType.mult,
                                    mybir.AluOpType.add,
                                )
                                ets.append(eti[:].bitcast(F16))
                            else:
                                et = etp.tile([128, QN], F16, tag="et")
                                nc.scalar.activation(et[:, :], st[:, :], exp_f)
                                ets.append(et[:])

                        # PV (accumulating over kblocks) + normalize
                        u = u_ps.tile([65, QN], F32, tag="u")
                        for b_ in range(kb):
                            nc.tensor.matmul(
                                u[:, :],
                                v_aug[:, (b_ * NH + h) * 66:(b_ * NH + h) * 66 + 65],
                                ets[b_][:, :],
                                start=(b_ == 0),
                                stop=(b_ == kb - 1),
                            )
                        # stage the denominator row to SBUF partition 0: the
                        # custom-DVE reciprocal mishandles PSUM/partition-64
                        # inputs on hardware (works in sim).
                        den = small.tile([1, QN], F32, tag="den")
                        nc.scalar.activation(den[:, :], u[64:65, :], ident_f, bias=0.0)
                        rec = small.tile([1, QN], F32, tag="rec")
                        nc.vector.reciprocal_approx_fast(rec[:, :], den[:, :])
                        if DEBUG_TAPS and j == 0 and h == 0:
                            nc.sync.dma_start(out=rec_dbg[:, :], in_=rec[:, :])
                            nc.sync.dma_start(out=et_dbg[:, :], in_=ets[0][:, :])
                        # broadcast 1/denom across partitions on the (idle)
                        # GpSimd engine; the DVE multiply then has a single
                        # PSUM operand (u) as required.
                        bcs = small.tile([64, QN], F32, tag="bcs")
                        nc.gpsimd.partition_broadcast(bcs[:, :], rec[0:1, :])
                        if DEBUG_TAPS and j == 0 and h == 0:
                            nc.sync.dma_start(out=bcs_dbg[:, :], in_=bcs[:, :])
                        xdst = (
                            xTA[j][h * 64:(h + 1) * 64, :]
                            if h < 2
                            else xTB[j][:, :]
                        )
                        nc.vector.tensor_mul(xdst, u[0:64, :], bcs[:, :])

                        if h == 0 and j > 0:
                            # emit phase O for the previous chunk here, after
                            # the next chunk's score matmuls are already
                            # queued: the PE then never sits idle waiting on
                            # the normalize chain (an idle window >3.4us
                            # triggers the 1.2GHz HAM throttle).
                            emit_o(j - 1)

                    if DEBUG_TAPS and j == 0:
                        nc.sync.dma_start(out=x_dbg[:, :], in_=xTA[0][:, :])
                        nc.sync.dma_start(out=qT_dbg[:, :], in_=qTp[:, :])
                        nc.sync.dma_start(out=kT_dbg[:, :], in_=kTp[:, :])
                        nc.sync.dma_start(out=va_dbg[:, :], in_=v_aug[:, :])
                emit_o(QC - 1)

    nc.compile()
    return nc


_PROGRAM_CACHE: dict[int, object] = {}


def _get_program(kb: int):
    if kb not in _PROGRAM_CACHE:
        _PROGRAM_CACHE[kb] = _build_program(kb)
    return _PROGRAM_CACHE[kb]


def _bf16(a: np.ndarray) -> np.ndarray:
    return np.ascontiguousarray(a).astype(np.float16)


def _pack_x(xt: np.ndarray, chunks) -> np.ndarray:
    """[D, L] -> [128, sum(DCH*w)] partition-major, chunked: [p,(c,dc,w)]."""
    parts = []
    for off, w in chunks:
        blk = xt[:, off:off + w].reshape(DCH, 128, w)
        parts.append(np.transpose(blk, (1, 0, 2)).reshape(128, DCH * w))
    return np.concatenate(parts, axis=1)


def _pack_w(wt: np.ndarray) -> np.ndarray:
    """[D, E] -> [128, DCH*E] partition-major: [p, (dc, e)]."""
    return np.transpose(wt.reshape(DCH, 128, E), (1, 0, 2)).reshape(128, DCH * E)


def _prep_inputs(query, key, value, mask, Wq, bq, Wk, bk, Wv, bv, Wo, bo):
    """Host-side shard prep. Returns (in_maps, kb)."""
    f32 = np.float32
    valid = [np.nonzero(mask[b, 0, 0, :] != 0)[0] for b in range(B)]
    s_valid = max((len(v) for v in valid), default=1)
    s_pad = max(128, -(-s_valid // 128) * 128)
    kb = s_pad // 128
    kch = _kchunks(s_pad)
    qch = [(i * QN, QN) for i in range(QC)]

    per_batch = []
    for b in range(B):
        vi = valid[b]
        xk_c = np.zeros((s_pad, D), dtype=f32)
        xv_c = np.zeros((s_pad, D), dtype=f32)
        xk_c[: len(vi)] = key[b][vi]
        xv_c[: len(vi)] = value[b][vi]
        # ones-column pattern: 1.0 for valid key rows, 0.0 for pad rows.
        vo = np.zeros((s_pad,), dtype=f32)
        vo[: len(vi)] = 1.0
        # [kblock*NH + h, kpos-within-block] -> [128, kb*NH, 2]
        # (second slot fills v_aug's alignment-pad column with zeros)
        vo_t = np.repeat(vo.reshape(kb, 1, 128), NH, axis=1).reshape(kb * NH, 128).T
        vo_t = np.stack([vo_t, np.zeros_like(vo_t)], axis=2).reshape(128, kb * NH * 2)
        per_batch.append(
            dict(
                xq_t=_bf16(_pack_x(query[b].T, qch)),
                xk_t=_bf16(_pack_x(xk_c.T, kch)),
                xv_t=_bf16(_pack_x(xv_c.T, kch)),
                vones=_bf16(vo_t),
            )
        )

    sc = f32(1.0 / np.sqrt(np.float32(DK)))
    in_maps = []
    for c in range(N_CORES):
        b = c // 4
        h0 = NH * (c % 4)
        sl = slice(h0 * DK, (h0 + NH) * DK)
        bqk_ = np.stack([bq[sl] * sc, bk[sl]], axis=1).astype(f32)
        in_maps.append(
            dict(
                per_batch[b],
                wq_t=_bf16(_pack_w(Wq[sl, :].T * sc)),
                wk_t=_bf16(_pack_w(Wk[sl, :].T)),
                wv_t=_bf16(_pack_w(Wv[sl, :].T)),
                wo_t=_bf16(Wo[:, sl].T),
                bqk=np.ascontiguousarray(bqk_),
            )
        )
    return in_maps, kb


def kernel(query, key, value, mask, Wq, bq, Wk, bk, Wv, bv, Wo, bo):
    from concourse.bass_utils import run_bass_kernel_spmd

    query = np.asarray(query, dtype=np.float32)
    key = np.asarray(key, dtype=np.float32)
    value = np.asarray(value, dtype=np.float32)
    mask = np.asarray(mask)
    Wq, Wk, Wv, Wo = (np.asarray(a, dtype=np.float32) for a in (Wq, Wk, Wv, Wo))
    bq, bk, bv, bo = (np.asarray(a, dtype=np.float32) for a in (bq, bk, bv, bo))

    in_maps, kb = _prep_inputs(
        query, key, value, mask, Wq, bq, Wk, bk, Wv, bv, Wo, bo
    )
    nc = _get_program(kb)
    res = run_bass_kernel_spmd(nc, in_maps, core_ids=list(range(N_CORES)))

    out = np.zeros((B, S, D), dtype=np.float32)
    for c in range(N_CORES):
        out[c // 4] += np.asarray(res.results[c]["out"], dtype=np.float32)
    # bv folds into the output as (sum_k p == 1) -> + bv @ Wo.T; bo is a plain
    # output bias. Both are zero for this problem's inputs; keep exactness for
    # any input without on-device cost.
    if np.any(bv) or np.any(bo):
        out += (bv @ Wo.T + bo)[None, None, :]
    return out
